# revision 22
# baseline (speedup 1.0000x reference)
"""AfterShockGNN (3-layer GCN + mean-pool + MLP head) on 8 Trainium2 NeuronCores.

Strategy
--------
Nodes are relabeled into a padded "position" space of NCORES*PPC rows with a few
reserved all-zero rows placed so that every gather index fits in int16:
the pre-aggregation feature table z is split into GROUPS of 32768 rows and each
edge indexes (src_pos % 32768) within its group's slice of the table.

Per layer:
  phase A (per core, own 12544 positions):  z = dinv * (h @ W)      (TensorE)
  AllGather z -> full table [P_TOT, 128] bf16 (row-padded to 256B)
  phase B: per-edge messages are pulled with dma_gather (256B rows), a one-hot
  scatter matrix P[edge, dst_slot] is built on VectorE (is_equal vs iota), and
  TensorE computes psum[dst_slot, feat] += P.T @ msg, accumulating all edges of
  a 128-position dst tile.  Bias enters as a rank-1 matmul (binv x b), and the
  flush applies  h = relu(dinv * psum)  in one fused tensor_scalar.

Pooling: psum[feat, graph] += h_tile.T @ S_tile (S = one-hot batch matrix),
AllReduce, then the 2-layer MLP head runs replicated on every core.
"""

import math

import numpy as np
import ml_dtypes

import concourse.bass as bass
import concourse.mybir as mybir
import concourse.tile as tile
from concourse import bacc, library_config
from concourse.bass_utils import run_bass_kernel_spmd

F32 = mybir.dt.float32
BF16 = mybir.dt.bfloat16
I16 = mybir.dt.int16
OP = mybir.AluOpType


class Cfg:
    def __init__(self, N, E, IN=128, H=64, OUT=2, G=64, TPC=98, GRP=32768):
        self.N, self.E, self.IN, self.H, self.OUT, self.G = N, E, IN, H, OUT, G
        self.NCORES = 8
        self.TPC = TPC                      # dst tiles (of 128 positions) per core
        self.PPC = 128 * TPC                # positions per core
        self.P_TOT = self.NCORES * self.PPC
        self.GRP = GRP                      # src-group size (int16 index reach)
        self.NG = -(-self.P_TOT // GRP)     # number of src groups
        assert self.NG <= 4
        # reserved all-zero positions: last row of each group
        self.reserved = sorted({g * GRP - 1 for g in range(1, self.NG)} | {self.P_TOT - 1})
        assert N <= self.P_TOT - len(self.reserved)
        self.ZPAD = 128                     # padded row width of z table (bf16 -> 256B)
        assert self.ZPAD >= H
        # halves for PSUM bank packing (<= 56 tiles -> 7 banks)
        nh = -(-TPC // 56)
        base = TPC // nh
        rem = TPC % nh
        self.halves = []
        t0 = 0
        for i in range(nh):
            n = base + (1 if i < rem else 0)
            self.halves.append((t0, t0 + n))
            t0 += n
        self.CALL_CC = 64                   # gather chunks (of 128 idxs) per dma_gather
        self.PW = 16                        # one-hot build window (chunks per DVE op)


REAL = Cfg(N=100000, E=1600000)


def _pos_map(cfg):
    """node id -> position, skipping reserved rows."""
    pos = np.arange(cfg.N, dtype=np.int64)
    for r in cfg.reserved:
        pos += pos >= r
    return pos


def prepare(x, edge_index, batch, cfg):
    """Host-side sharding/prep. Returns (sched, per_core_inputs: list[dict])."""
    N, E, G = cfg.N, cfg.E, cfg.G
    NC, PPC, TPC, NG, GRP = cfg.NCORES, cfg.PPC, cfg.TPC, cfg.NG, cfg.GRP

    x = np.asarray(x, dtype=np.float32)
    edge_index = np.asarray(edge_index, dtype=np.int64)
    batch = np.asarray(batch, dtype=np.int64)

    pos = _pos_map(cfg)
    # edges incl. self loops, in position space
    src = np.concatenate([pos[edge_index[0]], pos])
    dst = np.concatenate([pos[edge_index[1]], pos])

    deg = np.bincount(dst, minlength=cfg.P_TOT).astype(np.float64)
    dinv = np.where(deg > 0, 1.0 / np.sqrt(np.maximum(deg, 1)), 0.0).astype(np.float32)
    binv = np.where(deg > 0, np.sqrt(deg), 0.0).astype(np.float32)

    # --- per-core edge partition (by dst) and (tile, group) run structure ---
    core_of = dst // PPC
    t_loc = (dst % PPC) >> 7
    g_of = src // GRP
    dst_loc = dst & 127
    src_loc = (src - g_of * GRP).astype(np.int64)

    counts = np.zeros((NC, TPC, NG), dtype=np.int64)
    per_core = []
    for c in range(NC):
        m = core_of == c
        key = t_loc[m] * NG + g_of[m]
        counts[c] = np.bincount(key, minlength=TPC * NG).reshape(TPC, NG)
        per_core.append((key, src_loc[m], dst_loc[m]))

    nch = -(-counts.max(axis=0) // 128)     # [TPC, NG] chunks (128 idx) per run

    # --- global chunk schedule: for half, for g, for t in half ---
    chunk_sched = {}                        # (h,g) -> list of (t, k)
    run_chunk_start = np.zeros((TPC, NG), dtype=np.int64)
    cid = 0
    for h, (ta, tb) in enumerate(cfg.halves):
        for g in range(NG):
            lst = []
            for t in range(ta, tb):
                run_chunk_start[t, g] = cid
                for k in range(nch[t, g]):
                    lst.append((t, k))
                    cid += 1
            chunk_sched[(h, g)] = lst
    NCHUNK = cid
    NIDX = NCHUNK * 128

    # last (nonempty) chunk of each tile -> stop flag
    stop_chunk = np.full(TPC, -1, dtype=np.int64)
    for t in range(TPC):
        gs = [g for g in range(NG) if nch[t, g] > 0]
        if gs:
            stop_chunk[t] = run_chunk_start[t, gs[-1]] + nch[t, gs[-1]] - 1

    # call plan: (h,g) -> list of (chunk_id_start, n_chunks)
    calls = {}
    for h, (ta, tb) in enumerate(cfg.halves):
        for g in range(NG):
            n = len(chunk_sched[(h, g)])
            if n == 0:
                calls[(h, g)] = []
                continue
            c0 = run_chunk_start[ta, g]
            calls[(h, g)] = [(c0 + a, min(cfg.CALL_CC, n - a)) for a in range(0, n, cfg.CALL_CC)]

    # group dummy index (points at a reserved zero row, local to the group)
    g_hi = [min((g + 1) * GRP, cfg.P_TOT) for g in range(NG)]
    dummy = [g_hi[g] - 1 - g * GRP for g in range(NG)]

    # --- per-core flattened idx / dst tables in global chunk order ---
    batch_pos = np.full(cfg.P_TOT, -1, dtype=np.int64)
    batch_pos[pos] = batch
    cnt = np.bincount(batch, minlength=G).astype(np.float64)
    cntinv = (1.0 / np.maximum(cnt, 1.0)).astype(np.float32)

    x_pos = np.zeros((cfg.P_TOT, cfg.IN), dtype=np.float32)
    x_pos[pos] = x

    per_core_inputs = []
    for c in range(NC):
        key, sl, dl = per_core[c]
        order = np.argsort(key, kind="stable")
        key_s, sl_s, dl_s = key[order], sl[order], dl[order]
        cnts = np.bincount(key_s, minlength=TPC * NG).reshape(TPC, NG)
        run_start_edge = np.zeros(TPC * NG, dtype=np.int64)
        np.cumsum(cnts.reshape(-1)[:-1], out=run_start_edge[1:])
        rank = np.arange(len(key_s)) - run_start_edge[key_s]
        slot = run_chunk_start.reshape(-1)[key_s] * 128 + rank

        idx_arr = np.zeros(NIDX, dtype=np.int16)
        # fill dummies per (half, group) chunk region first
        for h in range(len(cfg.halves)):
            for g in range(NG):
                lst = chunk_sched[(h, g)]
                if lst:
                    a = run_chunk_start[lst[0][0], g] * 128
                    b = a + len(lst) * 128
                    idx_arr[a:b] = dummy[g]
        idx_arr[slot] = sl_s.astype(np.int16)
        dst_arr = np.zeros(NIDX, dtype=np.float32)
        dst_arr[slot] = dl_s
        # wrapped int16 idx layout [128, NIDX/16]
        idx_tab = np.tile(idx_arr.reshape(-1, 16).T, (8, 1)).astype(np.int16)
        dst_tab = dst_arr.reshape(NCHUNK, 128).T.astype(ml_dtypes.bfloat16)

        base = c * PPC
        xT = np.ascontiguousarray(x_pos[base:base + PPC].T).astype(ml_dtypes.bfloat16)
        dinv_tab = np.ascontiguousarray(dinv[base:base + PPC].reshape(TPC, 128).T)
        # per-bank bias lhsT: binv8[s, gb*128 + p] = binv at tile (bank, slice s)
        NBANK = sum(-(-(tb - ta) // 8) for (ta, tb) in cfg.halves)
        binv8 = np.zeros((8, NBANK * 128), dtype=np.float32)
        gb = 0
        for (ta, tb) in cfg.halves:
            for b in range(-(-(tb - ta) // 8)):
                for s in range(8):
                    t = ta + b * 8 + s
                    if t < tb:
                        binv8[s, gb * 128:(gb + 1) * 128] = \
                            binv[base + t * 128: base + (t + 1) * 128]
                gb += 1

        bp = batch_pos[base:base + PPC].reshape(TPC, 128)
        S = np.zeros((128, TPC, G), dtype=ml_dtypes.bfloat16)
        for t in range(TPC):
            valid = bp[t] >= 0
            S[valid, t, bp[t][valid]] = 1.0

        per_core_inputs.append(dict(
            xT=xT, idx=idx_tab, dstl=dst_tab,
            dinv=dinv_tab.astype(np.float32), binv8=binv8,
            S=S.reshape(128, TPC * G),
            cnti=np.broadcast_to(cntinv[None, :], (cfg.H, G)).copy().astype(np.float32),
        ))

    iota = np.broadcast_to(np.arange(128, dtype=np.float32)[None, None, :],
                           (128, cfg.PW, 128)).astype(ml_dtypes.bfloat16).copy()
    for d in per_core_inputs:
        d["iota"] = iota

    # per-bank last chunk (emission order = ascending cid) -> carries stop flag
    NBANK = sum(-(-(tb - ta) // 8) for (ta, tb) in cfg.halves)
    bank_of_tile = np.zeros(TPC, dtype=np.int64)
    bank_stop = np.full(NBANK, -1, dtype=np.int64)
    gb = 0
    for (ta, tb) in cfg.halves:
        for b in range(-(-(tb - ta) // 8)):
            for s in range(8):
                t = ta + b * 8 + s
                if t < tb:
                    bank_of_tile[t] = gb
                    bank_stop[gb] = max(bank_stop[gb], stop_chunk[t])
            gb += 1

    sched = dict(nch=nch, chunk_sched=chunk_sched, calls=calls, stop_chunk=stop_chunk,
                 run_chunk_start=run_chunk_start, NCHUNK=NCHUNK, NIDX=NIDX,
                 dummy=dummy, g_hi=g_hi, NBANK=NBANK, bank_of_tile=bank_of_tile,
                 bank_stop=bank_stop)
    return sched, per_core_inputs


def _blk_diag_b(b):
    b = np.asarray(b, np.float32)
    H = b.shape[0]
    out = np.zeros((8, 8 * H), dtype=np.float32)
    for s in range(8):
        out[s, s * H:(s + 1) * H] = b
    return out


def add_weight_inputs(per_core_inputs, W1, b1, W2, b2, W3, b3, Wm1, bm1, Wm2, bm2):
    w = dict(
        W1=np.asarray(W1, np.float32).astype(ml_dtypes.bfloat16),
        W2=np.asarray(W2, np.float32).astype(ml_dtypes.bfloat16),
        W3=np.asarray(W3, np.float32).astype(ml_dtypes.bfloat16),
        b1=_blk_diag_b(b1),
        b2=_blk_diag_b(b2),
        b3=_blk_diag_b(b3),
        Wm1=np.asarray(Wm1, np.float32),
        bm1=np.asarray(bm1, np.float32)[:, None],
        Wm2=np.asarray(Wm2, np.float32),
        bm2=np.asarray(bm2, np.float32)[:, None],
    )
    for d in per_core_inputs:
        d.update(w)


def build_program(cfg, sched):
    IN, H, OUT, G = cfg.IN, cfg.H, cfg.OUT, cfg.G
    TPC, PPC, NG = cfg.TPC, cfg.PPC, cfg.NG
    NCHUNK, NIDX = sched["NCHUNK"], sched["NIDX"]
    nch, stop_chunk = sched["nch"], sched["stop_chunk"]
    run_chunk_start = sched["run_chunk_start"]

    nc = bacc.Bacc("TRN2", target_bir_lowering=False, debug=False,
                   num_devices=cfg.NCORES)

    di = lambda n, s, d: nc.dram_tensor(n, s, d, kind="ExternalInput")
    xT_p = di("xT", [IN, PPC], BF16)
    idx_p = di("idx", [128, NIDX // 16], I16)
    dst_p = di("dstl", [128, NCHUNK], BF16)
    iota_p = di("iota", [128, cfg.PW, 128], BF16)
    dinv_p = di("dinv", [128, TPC], F32)
    binv_p = di("binv8", [8, sched["NBANK"] * 128], F32)
    S_p = di("S", [128, TPC * G], BF16)
    cnti_p = di("cnti", [H, G], F32)
    W_p = [di("W1", [IN, H], BF16), di("W2", [H, H], BF16), di("W3", [H, H], BF16)]
    b_p = [di("b1", [8, 8 * H], F32), di("b2", [8, 8 * H], F32), di("b3", [8, 8 * H], F32)]
    Wm1_p = di("Wm1", [H, H], F32)
    bm1_p = di("bm1", [H, 1], F32)
    Wm2_p = di("Wm2", [H, OUT], F32)
    bm2_p = di("bm2", [OUT, 1], F32)
    out_p = nc.dram_tensor("out", [OUT, G], F32, kind="ExternalOutput")
    dbg = getattr(cfg, "debug_taps", False)
    if dbg:
        zdbg_p = nc.dram_tensor("zdbg", [3, cfg.P_TOT, cfg.ZPAD], BF16,
                                kind="ExternalOutput")
        hdbg_p = nc.dram_tensor("hdbg", [3, 128, TPC, cfg.ZPAD], BF16,
                                kind="ExternalOutput")
        pdbg_p = nc.dram_tensor("pdbg", [H, G], F32, kind="ExternalOutput")

    z_full = nc.dram_tensor("z_full", [cfg.P_TOT, cfg.ZPAD], BF16, addr_space="Shared")
    pool_full = nc.dram_tensor("pool_full", [H, G], F32, addr_space="Shared")
    z_loc = nc.dram_tensor("z_loc", [PPC, cfg.ZPAD], BF16)
    h_dram = nc.dram_tensor("h_dram", [PPC, cfg.ZPAD], BF16)
    pool_loc = nc.dram_tensor("pool_loc", [H, G], F32)

    rg = [list(range(cfg.NCORES))]

    with tile.TileContext(nc) as tc:
        nc.gpsimd.load_library(library_config.mlp)
        tc.strict_bb_all_engine_barrier()

        import contextlib
        with contextlib.ExitStack() as ctx:
            cpool = ctx.enter_context(tc.tile_pool(name="consts", bufs=1))

            def load(p, shape, dt):
                t = cpool.tile(shape, dt, tag=p.name)
                nc.sync.dma_start(t[:], p[:])
                return t

            xT_sb = load(xT_p, [IN, PPC], BF16)
            idx_sb = load(idx_p, [128, NIDX // 16], I16)
            dst_sb = load(dst_p, [128, NCHUNK], BF16)
            iota_sb = load(iota_p, [128, cfg.PW, 128], BF16)
            dinv_sb = load(dinv_p, [128, TPC], F32)
            S_sb = load(S_p, [128, TPC * G], BF16)
            cnti_sb = load(cnti_p, [H, G], F32)
            W_sb = [load(W_p[i], [IN if i == 0 else H, H], BF16) for i in range(3)]
            b_sb = [load(b_p[i], [8, 8 * H], F32) for i in range(3)]
            binv_sb = load(binv_p, [8, sched["NBANK"] * 128], F32)
            Wm1_sb = load(Wm1_p, [H, H], F32)
            bm1_sb = load(bm1_p, [H, 1], F32)
            Wm2_sb = load(Wm2_p, [H, OUT], F32)
            bm2_sb = load(bm2_p, [OUT, 1], F32)

            zh = cpool.tile([128, TPC, cfg.ZPAD], BF16, tag="zh")    # z / h staging
            hT_sb = cpool.tile([128, PPC], BF16, tag="hT")
            nc.vector.memset(zh[:], 0.0)

            for L in range(3):
                K = IN if L == 0 else H
                # ---- phase A: z = dinv * (h @ W) ----
                with tc.tile_pool(name=f"psA{L}", bufs=4, space="PSUM") as psA:
                    for t in range(TPC):
                        ps = psA.tile([128, H], F32, tag="psA")
                        lhsT = (xT_sb if L == 0 else hT_sb)[0:K, t * 128:(t + 1) * 128]
                        nc.tensor.matmul(ps[:], lhsT, W_sb[L][:], start=True, stop=True)
                        nc.vector.tensor_scalar(
                            zh[:, t, 0:H], ps[:], dinv_sb[:, t:t + 1], None, OP.mult)
                nc.sync.dma_start(
                    out=z_loc.rearrange("(t p) c -> p t c", p=128), in_=zh[:])
                tc.strict_bb_all_engine_barrier()
                nc.gpsimd.collective_compute(
                    "AllGather", OP.bypass, replica_groups=rg,
                    ins=[z_loc[:]], outs=[z_full[:]])
                tc.strict_bb_all_engine_barrier()
                if dbg:
                    nc.sync.dma_start(out=zdbg_p[L], in_=z_full[:])

                # ---- phase B: aggregate ----
                with contextlib.ExitStack() as bctx:
                    psB = bctx.enter_context(
                        tc.tile_pool(name=f"psB{L}", bufs=8, space="PSUM"))
                    mpool = bctx.enter_context(
                        tc.tile_pool(name=f"msg{L}", bufs=3))
                    ppool = bctx.enter_context(
                        tc.tile_pool(name=f"P{L}", bufs=3))
                    bank_base = 0
                    for h, (ta, tb) in enumerate(cfg.halves):
                        if h > 0:
                            bank_base += -(-(cfg.halves[h-1][1] - cfg.halves[h-1][0]) // 8)
                        nbank = -(-(tb - ta) // 8)
                        banks = [psB.tile([128, 512], F32, tag="bank",
                                          name=f"bank_L{L}_h{h}_{i}")
                                 for i in range(nbank)]

                        def bank_ap(t):
                            tl = t - ta
                            return banks[tl // 8][:, (tl % 8) * H:(tl % 8) * H + H]

                        for b in range(nbank):
                            gb = bank_base + b
                            nc.tensor.matmul(
                                banks[b][:, 0:8 * H],
                                binv_sb[:, gb * 128:(gb + 1) * 128],
                                b_sb[L][:], start=True,
                                stop=(sched["bank_stop"][gb] < 0))
                        for g in range(NG):
                            glo = g * cfg.GRP
                            z_src = z_full[glo:sched["g_hi"][g], :]
                            for (c0, ncc) in sched["calls"][(h, g)]:
                                msg = mpool.tile([128, cfg.CALL_CC, cfg.ZPAD], BF16, tag="msg")
                                nidx = ncc * 128
                                nc.gpsimd.dma_gather(
                                    msg[:, 0:ncc, :], z_src,
                                    idx_sb[:, c0 * 8:(c0 + ncc) * 8],
                                    nidx, nidx, cfg.ZPAD,
                                    single_packet=False)
                                for w0 in range(0, ncc, cfg.PW):
                                    w = min(cfg.PW, ncc - w0)
                                    P = ppool.tile([128, cfg.PW, 128], BF16, tag="P")
                                    nc.vector.tensor_tensor(
                                        P[:, 0:w, :], iota_sb[:, 0:w, :],
                                        dst_sb[:, c0 + w0:c0 + w0 + w, None]
                                        .to_broadcast((128, w, 128)),
                                        OP.is_equal)
                                    for j in range(w):
                                        cid = c0 + w0 + j
                                        t, k = sched["chunk_sched"][(h, g)][cid - run_chunk_start[ta, g]]
                                        gb = sched["bank_of_tile"][t]
                                        nc.tensor.matmul(
                                            bank_ap(t), P[:, j, :],
                                            msg[:, w0 + j, 0:H],
                                            start=False,
                                            stop=(cid == sched["bank_stop"][gb]))
                        for t in range(ta, tb):
                            nc.vector.tensor_scalar(
                                zh[:, t, 0:H], bank_ap(t),
                                dinv_sb[:, t:t + 1], 0.0, OP.mult, OP.max)

                if dbg:
                    nc.sync.dma_start(out=hdbg_p[L], in_=zh[:])
                if L < 2:
                    nc.sync.dma_start(
                        out=h_dram.rearrange("(t p) c -> p t c", p=128), in_=zh[:])
                    tc.strict_bb_all_engine_barrier()
                    nc.sync.dma_start_transpose(out=hT_sb[:], in_=h_dram[:])

            # ---- mean pooling ----
            with tc.tile_pool(name="psP", bufs=2, space="PSUM") as psP:
                pp = psP.tile([H, G], F32, tag="pp")
                for t in range(TPC):
                    nc.tensor.matmul(
                        pp[:], zh[:, t, 0:H], S_sb[:, t * G:(t + 1) * G],
                        start=(t == 0), stop=(t == TPC - 1))
                pool_sb = cpool.tile([H, G], F32, tag="pool_sb")
                nc.vector.tensor_copy(pool_sb[:], pp[:])
            nc.sync.dma_start(out=pool_loc[:], in_=pool_sb[:])
            if dbg:
                nc.sync.dma_start(out=pdbg_p[:], in_=pool_sb[:])
            tc.strict_bb_all_engine_barrier()
            nc.gpsimd.collective_compute(
                "AllReduce", OP.add, replica_groups=rg,
                ins=[pool_loc[:]], outs=[pool_full[:]])
            tc.strict_bb_all_engine_barrier()

            # ---- MLP head (replicated) ----
            with tc.tile_pool(name="psM", bufs=2, space="PSUM") as psM:
                pooled = cpool.tile([H, G], F32, tag="pooled")
                nc.sync.dma_start(pooled[:], pool_full[:])
                pooln = cpool.tile([H, G], F32, tag="pooln")
                nc.vector.tensor_tensor(pooln[:], pooled[:], cnti_sb[:], OP.mult)
                ps1 = psM.tile([H, G], F32, tag="ps1")
                nc.tensor.matmul(ps1[:], Wm1_sb[:], pooln[:], start=True, stop=True)
                z1 = cpool.tile([H, G], F32, tag="z1")
                nc.scalar.activation(z1[:], ps1[:],
                                     mybir.ActivationFunctionType.Relu,
                                     bias=bm1_sb[:, 0:1], scale=1.0)
                ps2 = psM.tile([OUT, G], F32, tag="ps2")
                nc.tensor.matmul(ps2[:], Wm2_sb[:], z1[:], start=True, stop=True)
                out_sb = cpool.tile([OUT, G], F32, tag="out_sb")
                nc.vector.tensor_scalar(out_sb[:], ps2[:], bm2_sb[:, 0:1], None, OP.add)
            nc.sync.dma_start(out=out_p[:], in_=out_sb[:])

    nc.compile()
    return nc


def run(inputs, cfg=REAL, via="hw", trace=False):
    """inputs: the reference.setup_inputs() dict (numpy). Returns [G, OUT] fp32."""
    sched, pci = prepare(inputs["x"], inputs["edge_index"], inputs["batch"], cfg)
    add_weight_inputs(pci, inputs["W1"], inputs["b1"], inputs["W2"], inputs["b2"],
                      inputs["W3"], inputs["b3"], inputs["Wm1"], inputs["bm1"],
                      inputs["Wm2"], inputs["bm2"])
    nc = build_program(cfg, sched)
    if via == "sim":
        from concourse.bass_interp import MultiCoreSim
        sim = MultiCoreSim(nc, num_cores=cfg.NCORES, require_finite=False,
                           require_nnan=False)
        for c, core in sim.cores.items():
            for k, v in pci[c].items():
                core.tensor(k)[:] = v
        sim.simulate(check_with_hw=False)
        out = np.array(sim.cores[0].tensor("out"))
        taps = None
        if getattr(cfg, "debug_taps", False):
            taps = {k: {c: np.array(sim.cores[c].tensor(k)) for c in sim.cores}
                    for k in ("zdbg", "hdbg", "pdbg")}
        return out.T.copy(), taps
    br = run_bass_kernel_spmd(nc, pci, list(range(cfg.NCORES)), trace=trace)
    out = br.results[0]["out"]
    return np.asarray(out, np.float32).T.copy(), br


def kernel(**inputs):
    inputs = {k: np.asarray(v) for k, v in inputs.items()}
    out, _ = run(inputs, REAL, via="hw")
    return out


# revision 27
# speedup vs baseline: 1.7519x; 1.7519x over previous
"""AfterShockGNN (3-layer GCN + mean-pool + MLP head) on 8 Trainium2 NeuronCores.

Strategy
--------
Nodes are relabeled into a padded "position" space of NCORES*PPC rows with a few
reserved all-zero rows placed so that every gather index fits in int16:
the pre-aggregation feature table z is split into GROUPS of 32768 rows and each
edge indexes (src_pos % 32768) within its group's slice of the table.

Per layer:
  phase A (per core, own 12544 positions):  z = dinv * (h @ W)      (TensorE)
  AllGather z -> full table [P_TOT, 128] bf16 (row-padded to 256B)
  phase B: per-edge messages are pulled with dma_gather (256B rows), a one-hot
  scatter matrix P[edge, dst_slot] is built on VectorE (is_equal vs iota), and
  TensorE computes psum[dst_slot, feat] += P.T @ msg, accumulating all edges of
  a 128-position dst tile.  Bias enters as a rank-1 matmul (binv x b), and the
  flush applies  h = relu(dinv * psum)  in one fused tensor_scalar.

Pooling: psum[feat, graph] += h_tile.T @ S_tile (S = one-hot batch matrix),
AllReduce, then the 2-layer MLP head runs replicated on every core.
"""

import math

import numpy as np
import ml_dtypes

import concourse.bass as bass
import concourse.mybir as mybir
import concourse.tile as tile
from concourse import bacc, library_config
from concourse.bass_utils import run_bass_kernel_spmd

F32 = mybir.dt.float32
BF16 = mybir.dt.bfloat16
I16 = mybir.dt.int16
OP = mybir.AluOpType


class Cfg:
    def __init__(self, N, E, IN=128, H=64, OUT=2, G=64, TPC=98, GRP=32768):
        self.N, self.E, self.IN, self.H, self.OUT, self.G = N, E, IN, H, OUT, G
        self.NCORES = 8
        self.TPC = TPC                      # dst tiles (of 128 positions) per core
        self.PPC = 128 * TPC                # positions per core
        self.P_TOT = self.NCORES * self.PPC
        self.GRP = GRP                      # src-group size (int16 index reach)
        self.NG = -(-self.P_TOT // GRP)     # number of src groups
        assert self.NG <= 4
        # reserved all-zero positions: last row of each group
        self.reserved = sorted({g * GRP - 1 for g in range(1, self.NG)} | {self.P_TOT - 1})
        assert N <= self.P_TOT - len(self.reserved)
        self.ZPAD = 128                     # padded row width of z table (bf16 -> 256B)
        assert self.ZPAD >= H
        # halves for PSUM bank packing (<= 56 tiles -> 7 banks)
        nh = -(-TPC // 56)
        base = TPC // nh
        rem = TPC % nh
        self.halves = []
        t0 = 0
        for i in range(nh):
            n = base + (1 if i < rem else 0)
            self.halves.append((t0, t0 + n))
            t0 += n
        self.CALL_CC = 32                   # gather chunks (of 128 idxs) per dma_gather
        self.PW = 16                        # one-hot build window (chunks per DVE op)
        self.NQ = 4                         # SWDGE queues (parallel desc rings)
        self.MBUFS = 4                      # message-tile buffers in flight


REAL = Cfg(N=100000, E=1600000)


def _pos_map(cfg):
    """node id -> position, skipping reserved rows."""
    pos = np.arange(cfg.N, dtype=np.int64)
    for r in cfg.reserved:
        pos += pos >= r
    return pos


def prepare(x, edge_index, batch, cfg):
    """Host-side sharding/prep. Returns (sched, per_core_inputs: list[dict])."""
    N, E, G = cfg.N, cfg.E, cfg.G
    NC, PPC, TPC, NG, GRP = cfg.NCORES, cfg.PPC, cfg.TPC, cfg.NG, cfg.GRP

    x = np.asarray(x, dtype=np.float32)
    edge_index = np.asarray(edge_index, dtype=np.int64)
    batch = np.asarray(batch, dtype=np.int64)

    pos = _pos_map(cfg)
    # edges incl. self loops, in position space
    src = np.concatenate([pos[edge_index[0]], pos])
    dst = np.concatenate([pos[edge_index[1]], pos])

    deg = np.bincount(dst, minlength=cfg.P_TOT).astype(np.float64)
    dinv = np.where(deg > 0, 1.0 / np.sqrt(np.maximum(deg, 1)), 0.0).astype(np.float32)
    binv = np.where(deg > 0, np.sqrt(deg), 0.0).astype(np.float32)

    # --- per-core edge partition (by dst) and (tile, group) run structure ---
    core_of = dst // PPC
    t_loc = (dst % PPC) >> 7
    g_of = src // GRP
    dst_loc = dst & 127
    src_loc = (src - g_of * GRP).astype(np.int64)

    counts = np.zeros((NC, TPC, NG), dtype=np.int64)
    per_core = []
    for c in range(NC):
        m = core_of == c
        key = t_loc[m] * NG + g_of[m]
        counts[c] = np.bincount(key, minlength=TPC * NG).reshape(TPC, NG)
        per_core.append((key, src_loc[m], dst_loc[m]))

    nch = -(-counts.max(axis=0) // 128)     # [TPC, NG] chunks (128 idx) per run

    # --- global chunk schedule: for half, for g, for t in half ---
    chunk_sched = {}                        # (h,g) -> list of (t, k)
    run_chunk_start = np.zeros((TPC, NG), dtype=np.int64)
    cid = 0
    for h, (ta, tb) in enumerate(cfg.halves):
        for g in range(NG):
            lst = []
            for t in range(ta, tb):
                run_chunk_start[t, g] = cid
                for k in range(nch[t, g]):
                    lst.append((t, k))
                    cid += 1
            chunk_sched[(h, g)] = lst
    NCHUNK = cid
    NIDX = NCHUNK * 128

    # last (nonempty) chunk of each tile -> stop flag
    stop_chunk = np.full(TPC, -1, dtype=np.int64)
    for t in range(TPC):
        gs = [g for g in range(NG) if nch[t, g] > 0]
        if gs:
            stop_chunk[t] = run_chunk_start[t, gs[-1]] + nch[t, gs[-1]] - 1

    # call plan: (h,g) -> list of (chunk_id_start, n_chunks)
    calls = {}
    for h, (ta, tb) in enumerate(cfg.halves):
        for g in range(NG):
            n = len(chunk_sched[(h, g)])
            if n == 0:
                calls[(h, g)] = []
                continue
            c0 = run_chunk_start[ta, g]
            calls[(h, g)] = [(c0 + a, min(cfg.CALL_CC, n - a)) for a in range(0, n, cfg.CALL_CC)]

    # group dummy index (points at a reserved zero row, local to the group)
    g_hi = [min((g + 1) * GRP, cfg.P_TOT) for g in range(NG)]
    dummy = [g_hi[g] - 1 - g * GRP for g in range(NG)]

    # --- per-core flattened idx / dst tables in global chunk order ---
    batch_pos = np.full(cfg.P_TOT, -1, dtype=np.int64)
    batch_pos[pos] = batch
    cnt = np.bincount(batch, minlength=G).astype(np.float64)
    cntinv = (1.0 / np.maximum(cnt, 1.0)).astype(np.float32)

    x_pos = np.zeros((cfg.P_TOT, cfg.IN), dtype=np.float32)
    x_pos[pos] = x

    per_core_inputs = []
    for c in range(NC):
        key, sl, dl = per_core[c]
        order = np.argsort(key, kind="stable")
        key_s, sl_s, dl_s = key[order], sl[order], dl[order]
        cnts = np.bincount(key_s, minlength=TPC * NG).reshape(TPC, NG)
        run_start_edge = np.zeros(TPC * NG, dtype=np.int64)
        np.cumsum(cnts.reshape(-1)[:-1], out=run_start_edge[1:])
        rank = np.arange(len(key_s)) - run_start_edge[key_s]
        slot = run_chunk_start.reshape(-1)[key_s] * 128 + rank

        idx_arr = np.zeros(NIDX, dtype=np.int16)
        # fill dummies per (half, group) chunk region first
        for h in range(len(cfg.halves)):
            for g in range(NG):
                lst = chunk_sched[(h, g)]
                if lst:
                    a = run_chunk_start[lst[0][0], g] * 128
                    b = a + len(lst) * 128
                    idx_arr[a:b] = dummy[g]
        idx_arr[slot] = sl_s.astype(np.int16)
        dst_arr = np.zeros(NIDX, dtype=np.float32)
        dst_arr[slot] = dl_s
        # wrapped int16 idx layout [128, NIDX/16]
        idx_tab = np.tile(idx_arr.reshape(-1, 16).T, (8, 1)).astype(np.int16)
        dst_tab = dst_arr.reshape(NCHUNK, 128).T.astype(ml_dtypes.bfloat16)

        base = c * PPC
        xT = np.ascontiguousarray(x_pos[base:base + PPC].T).astype(ml_dtypes.bfloat16)
        dinv_tab = np.ascontiguousarray(dinv[base:base + PPC].reshape(TPC, 128).T)
        # per-bank bias lhsT: binv8[s, gb*128 + p] = binv at tile (bank, slice s)
        NBANK = sum(-(-(tb - ta) // 8) for (ta, tb) in cfg.halves)
        binv8 = np.zeros((8, NBANK * 128), dtype=np.float32)
        gb = 0
        for (ta, tb) in cfg.halves:
            for b in range(-(-(tb - ta) // 8)):
                for s in range(8):
                    t = ta + b * 8 + s
                    if t < tb:
                        binv8[s, gb * 128:(gb + 1) * 128] = \
                            binv[base + t * 128: base + (t + 1) * 128]
                gb += 1

        bp = batch_pos[base:base + PPC].reshape(TPC, 128)
        S = np.zeros((128, TPC, G), dtype=ml_dtypes.bfloat16)
        for t in range(TPC):
            valid = bp[t] >= 0
            S[valid, t, bp[t][valid]] = 1.0

        per_core_inputs.append(dict(
            xT=xT, idx=idx_tab, dstl=dst_tab,
            dinv=dinv_tab.astype(np.float32), binv8=binv8,
            S=S.reshape(128, TPC * G),
            cnti=np.broadcast_to(cntinv[None, :], (cfg.H, G)).copy().astype(np.float32),
        ))

    iota = np.broadcast_to(np.arange(128, dtype=np.float32)[None, None, :],
                           (128, cfg.PW, 128)).astype(ml_dtypes.bfloat16).copy()
    for d in per_core_inputs:
        d["iota"] = iota

    # per-bank last chunk (emission order = ascending cid) -> carries stop flag
    NBANK = sum(-(-(tb - ta) // 8) for (ta, tb) in cfg.halves)
    bank_of_tile = np.zeros(TPC, dtype=np.int64)
    bank_stop = np.full(NBANK, -1, dtype=np.int64)
    gb = 0
    for (ta, tb) in cfg.halves:
        for b in range(-(-(tb - ta) // 8)):
            for s in range(8):
                t = ta + b * 8 + s
                if t < tb:
                    bank_of_tile[t] = gb
                    bank_stop[gb] = max(bank_stop[gb], stop_chunk[t])
            gb += 1

    sched = dict(nch=nch, chunk_sched=chunk_sched, calls=calls, stop_chunk=stop_chunk,
                 run_chunk_start=run_chunk_start, NCHUNK=NCHUNK, NIDX=NIDX,
                 dummy=dummy, g_hi=g_hi, NBANK=NBANK, bank_of_tile=bank_of_tile,
                 bank_stop=bank_stop)
    return sched, per_core_inputs


def _blk_diag_b(b):
    b = np.asarray(b, np.float32)
    H = b.shape[0]
    out = np.zeros((8, 8 * H), dtype=np.float32)
    for s in range(8):
        out[s, s * H:(s + 1) * H] = b
    return out


def add_weight_inputs(per_core_inputs, W1, b1, W2, b2, W3, b3, Wm1, bm1, Wm2, bm2):
    w = dict(
        W1=np.asarray(W1, np.float32).astype(ml_dtypes.bfloat16),
        W2=np.asarray(W2, np.float32).astype(ml_dtypes.bfloat16),
        W3=np.asarray(W3, np.float32).astype(ml_dtypes.bfloat16),
        b1=_blk_diag_b(b1),
        b2=_blk_diag_b(b2),
        b3=_blk_diag_b(b3),
        Wm1=np.asarray(Wm1, np.float32),
        bm1=np.asarray(bm1, np.float32)[:, None],
        Wm2=np.asarray(Wm2, np.float32),
        bm2=np.asarray(bm2, np.float32)[:, None],
    )
    for d in per_core_inputs:
        d.update(w)


def build_program(cfg, sched):
    IN, H, OUT, G = cfg.IN, cfg.H, cfg.OUT, cfg.G
    TPC, PPC, NG = cfg.TPC, cfg.PPC, cfg.NG
    NCHUNK, NIDX = sched["NCHUNK"], sched["NIDX"]
    nch, stop_chunk = sched["nch"], sched["stop_chunk"]
    run_chunk_start = sched["run_chunk_start"]

    nc = bacc.Bacc("TRN2", target_bir_lowering=False, debug=False,
                   num_devices=cfg.NCORES,
                   num_swdge_queues=getattr(cfg, "NQ", 1))

    di = lambda n, s, d: nc.dram_tensor(n, s, d, kind="ExternalInput")
    xT_p = di("xT", [IN, PPC], BF16)
    idx_p = di("idx", [128, NIDX // 16], I16)
    dst_p = di("dstl", [128, NCHUNK], BF16)
    iota_p = di("iota", [128, cfg.PW, 128], BF16)
    dinv_p = di("dinv", [128, TPC], F32)
    binv_p = di("binv8", [8, sched["NBANK"] * 128], F32)
    S_p = di("S", [128, TPC * G], BF16)
    cnti_p = di("cnti", [H, G], F32)
    W_p = [di("W1", [IN, H], BF16), di("W2", [H, H], BF16), di("W3", [H, H], BF16)]
    b_p = [di("b1", [8, 8 * H], F32), di("b2", [8, 8 * H], F32), di("b3", [8, 8 * H], F32)]
    Wm1_p = di("Wm1", [H, H], F32)
    bm1_p = di("bm1", [H, 1], F32)
    Wm2_p = di("Wm2", [H, OUT], F32)
    bm2_p = di("bm2", [OUT, 1], F32)
    out_p = nc.dram_tensor("out", [OUT, G], F32, kind="ExternalOutput")
    dbg = getattr(cfg, "debug_taps", False)
    if dbg:
        zdbg_p = nc.dram_tensor("zdbg", [3, cfg.P_TOT, cfg.ZPAD], BF16,
                                kind="ExternalOutput")
        hdbg_p = nc.dram_tensor("hdbg", [3, 128, TPC, cfg.ZPAD], BF16,
                                kind="ExternalOutput")
        pdbg_p = nc.dram_tensor("pdbg", [H, G], F32, kind="ExternalOutput")

    z_full = nc.dram_tensor("z_full", [cfg.P_TOT, cfg.ZPAD], BF16, addr_space="Shared")
    pool_full = nc.dram_tensor("pool_full", [H, G], F32, addr_space="Shared")
    z_loc = nc.dram_tensor("z_loc", [PPC, cfg.ZPAD], BF16)
    h_dram = nc.dram_tensor("h_dram", [PPC, cfg.ZPAD], BF16)
    pool_loc = nc.dram_tensor("pool_loc", [H, G], F32)

    rg = [list(range(cfg.NCORES))]

    with tile.TileContext(nc) as tc:
        nc.gpsimd.load_library(library_config.mlp)
        tc.strict_bb_all_engine_barrier()

        import contextlib
        with contextlib.ExitStack() as ctx:
            cpool = ctx.enter_context(tc.tile_pool(name="consts", bufs=1))

            def load(p, shape, dt):
                t = cpool.tile(shape, dt, tag=p.name)
                nc.sync.dma_start(t[:], p[:])
                return t

            xT_sb = load(xT_p, [IN, PPC], BF16)
            idx_sb = load(idx_p, [128, NIDX // 16], I16)
            dst_sb = load(dst_p, [128, NCHUNK], BF16)
            iota_sb = load(iota_p, [128, cfg.PW, 128], BF16)
            dinv_sb = load(dinv_p, [128, TPC], F32)
            S_sb = load(S_p, [128, TPC * G], BF16)
            cnti_sb = load(cnti_p, [H, G], F32)
            W_sb = [load(W_p[i], [IN if i == 0 else H, H], BF16) for i in range(3)]
            b_sb = [load(b_p[i], [8, 8 * H], F32) for i in range(3)]
            binv_sb = load(binv_p, [8, sched["NBANK"] * 128], F32)
            Wm1_sb = load(Wm1_p, [H, H], F32)
            bm1_sb = load(bm1_p, [H, 1], F32)
            Wm2_sb = load(Wm2_p, [H, OUT], F32)
            bm2_sb = load(bm2_p, [OUT, 1], F32)

            zh = cpool.tile([128, TPC, cfg.ZPAD], BF16, tag="zh")    # z / h staging
            hT_sb = cpool.tile([128, PPC], BF16, tag="hT")
            nc.vector.memset(zh[:], 0.0)

            for L in range(3):
                K = IN if L == 0 else H
                # ---- phase A: z = dinv * (h @ W) ----
                with tc.tile_pool(name=f"psA{L}", bufs=4, space="PSUM") as psA:
                    for t in range(TPC):
                        ps = psA.tile([128, H], F32, tag="psA")
                        lhsT = (xT_sb if L == 0 else hT_sb)[0:K, t * 128:(t + 1) * 128]
                        nc.tensor.matmul(ps[:], lhsT, W_sb[L][:], start=True, stop=True)
                        nc.vector.tensor_scalar(
                            zh[:, t, 0:H], ps[:], dinv_sb[:, t:t + 1], None, OP.mult)
                nc.sync.dma_start(
                    out=z_loc.rearrange("(t p) c -> p t c", p=128), in_=zh[:])
                tc.strict_bb_all_engine_barrier()
                nc.gpsimd.collective_compute(
                    "AllGather", OP.bypass, replica_groups=rg,
                    ins=[z_loc[:]], outs=[z_full[:]])
                tc.strict_bb_all_engine_barrier()
                if dbg:
                    nc.sync.dma_start(out=zdbg_p[L], in_=z_full[:])

                # ---- phase B: aggregate ----
                with contextlib.ExitStack() as bctx:
                    psB = bctx.enter_context(
                        tc.tile_pool(name=f"psB{L}", bufs=8, space="PSUM"))
                    mpool = bctx.enter_context(
                        tc.tile_pool(name=f"msg{L}", bufs=getattr(cfg, "MBUFS", 3)))
                    ppool = bctx.enter_context(
                        tc.tile_pool(name=f"P{L}", bufs=3))
                    bank_base = 0
                    for h, (ta, tb) in enumerate(cfg.halves):
                        if h > 0:
                            bank_base += -(-(cfg.halves[h-1][1] - cfg.halves[h-1][0]) // 8)
                        nbank = -(-(tb - ta) // 8)
                        banks = [psB.tile([128, 512], F32, tag="bank",
                                          name=f"bank_L{L}_h{h}_{i}")
                                 for i in range(nbank)]

                        def bank_ap(t):
                            tl = t - ta
                            return banks[tl // 8][:, (tl % 8) * H:(tl % 8) * H + H]

                        for b in range(nbank):
                            gb = bank_base + b
                            nc.tensor.matmul(
                                banks[b][:, 0:8 * H],
                                binv_sb[:, gb * 128:(gb + 1) * 128],
                                b_sb[L][:], start=True,
                                stop=(sched["bank_stop"][gb] < 0))
                        qn = 0
                        for g in range(NG):
                            glo = g * cfg.GRP
                            z_src = z_full[glo:sched["g_hi"][g], :]
                            for (c0, ncc) in sched["calls"][(h, g)]:
                                msg = mpool.tile([128, cfg.CALL_CC, cfg.ZPAD], BF16, tag="msg")
                                nidx = ncc * 128
                                if getattr(cfg, "ablate", None) != "no_gather":
                                    nc.gpsimd.dma_gather(
                                        msg[:, 0:ncc, :], z_src,
                                        idx_sb[:, c0 * 8:(c0 + ncc) * 8],
                                        nidx, nidx, cfg.ZPAD,
                                        single_packet=False,
                                        queue_num=qn % getattr(cfg, "NQ", 1))
                                    qn += 1
                                if getattr(cfg, "ablate", None) == "gather_only":
                                    continue
                                for w0 in range(0, ncc, cfg.PW):
                                    w = min(cfg.PW, ncc - w0)
                                    P = ppool.tile([128, cfg.PW, 128], BF16, tag="P")
                                    nc.vector.tensor_tensor(
                                        P[:, 0:w, :], iota_sb[:, 0:w, :],
                                        dst_sb[:, c0 + w0:c0 + w0 + w, None]
                                        .to_broadcast((128, w, 128)),
                                        OP.is_equal)
                                    for j in range(w):
                                        cid = c0 + w0 + j
                                        t, k = sched["chunk_sched"][(h, g)][cid - run_chunk_start[ta, g]]
                                        gb = sched["bank_of_tile"][t]
                                        nc.tensor.matmul(
                                            bank_ap(t), P[:, j, :],
                                            msg[:, w0 + j, 0:H],
                                            start=False,
                                            stop=(cid == sched["bank_stop"][gb]))
                        for t in range(ta, tb):
                            nc.vector.tensor_scalar(
                                zh[:, t, 0:H], bank_ap(t),
                                dinv_sb[:, t:t + 1], 0.0, OP.mult, OP.max)

                if dbg:
                    nc.sync.dma_start(out=hdbg_p[L], in_=zh[:])
                if L < 2:
                    nc.sync.dma_start(
                        out=h_dram.rearrange("(t p) c -> p t c", p=128), in_=zh[:])
                    tc.strict_bb_all_engine_barrier()
                    nc.sync.dma_start_transpose(out=hT_sb[:], in_=h_dram[:])

            # ---- mean pooling ----
            with tc.tile_pool(name="psP", bufs=2, space="PSUM") as psP:
                pp = psP.tile([H, G], F32, tag="pp")
                for t in range(TPC):
                    nc.tensor.matmul(
                        pp[:], zh[:, t, 0:H], S_sb[:, t * G:(t + 1) * G],
                        start=(t == 0), stop=(t == TPC - 1))
                pool_sb = cpool.tile([H, G], F32, tag="pool_sb")
                nc.vector.tensor_copy(pool_sb[:], pp[:])
            nc.sync.dma_start(out=pool_loc[:], in_=pool_sb[:])
            if dbg:
                nc.sync.dma_start(out=pdbg_p[:], in_=pool_sb[:])
            tc.strict_bb_all_engine_barrier()
            nc.gpsimd.collective_compute(
                "AllReduce", OP.add, replica_groups=rg,
                ins=[pool_loc[:]], outs=[pool_full[:]])
            tc.strict_bb_all_engine_barrier()

            # ---- MLP head (replicated) ----
            with tc.tile_pool(name="psM", bufs=2, space="PSUM") as psM:
                pooled = cpool.tile([H, G], F32, tag="pooled")
                nc.sync.dma_start(pooled[:], pool_full[:])
                pooln = cpool.tile([H, G], F32, tag="pooln")
                nc.vector.tensor_tensor(pooln[:], pooled[:], cnti_sb[:], OP.mult)
                ps1 = psM.tile([H, G], F32, tag="ps1")
                nc.tensor.matmul(ps1[:], Wm1_sb[:], pooln[:], start=True, stop=True)
                z1 = cpool.tile([H, G], F32, tag="z1")
                nc.scalar.activation(z1[:], ps1[:],
                                     mybir.ActivationFunctionType.Relu,
                                     bias=bm1_sb[:, 0:1], scale=1.0)
                ps2 = psM.tile([OUT, G], F32, tag="ps2")
                nc.tensor.matmul(ps2[:], Wm2_sb[:], z1[:], start=True, stop=True)
                out_sb = cpool.tile([OUT, G], F32, tag="out_sb")
                nc.vector.tensor_scalar(out_sb[:], ps2[:], bm2_sb[:, 0:1], None, OP.add)
            nc.sync.dma_start(out=out_p[:], in_=out_sb[:])

    nc.compile()
    return nc


def run(inputs, cfg=REAL, via="hw", trace=False):
    """inputs: the reference.setup_inputs() dict (numpy). Returns [G, OUT] fp32."""
    sched, pci = prepare(inputs["x"], inputs["edge_index"], inputs["batch"], cfg)
    add_weight_inputs(pci, inputs["W1"], inputs["b1"], inputs["W2"], inputs["b2"],
                      inputs["W3"], inputs["b3"], inputs["Wm1"], inputs["bm1"],
                      inputs["Wm2"], inputs["bm2"])
    nc = build_program(cfg, sched)
    if via == "sim":
        from concourse.bass_interp import MultiCoreSim
        sim = MultiCoreSim(nc, num_cores=cfg.NCORES, require_finite=False,
                           require_nnan=False)
        for c, core in sim.cores.items():
            for k, v in pci[c].items():
                core.tensor(k)[:] = v
        sim.simulate(check_with_hw=False)
        out = np.array(sim.cores[0].tensor("out"))
        taps = None
        if getattr(cfg, "debug_taps", False):
            taps = {k: {c: np.array(sim.cores[c].tensor(k)) for c in sim.cores}
                    for k in ("zdbg", "hdbg", "pdbg")}
        return out.T.copy(), taps
    br = run_bass_kernel_spmd(nc, pci, list(range(cfg.NCORES)), trace=trace)
    out = br.results[0]["out"]
    return np.asarray(out, np.float32).T.copy(), br


def kernel(**inputs):
    inputs = {k: np.asarray(v) for k, v in inputs.items()}
    out, _ = run(inputs, REAL, via="hw")
    return out


# revision 30
# speedup vs baseline: 4.6401x; 2.6486x over previous
"""AfterShockGNN (3-layer GCN + mean-pool + MLP head) on 8 Trainium2 NeuronCores.

Strategy
--------
Nodes are relabeled into a padded "position" space of NCORES*PPC rows with a few
reserved all-zero rows placed so that every gather index fits in int16:
the pre-aggregation feature table z is split into GROUPS of 32768 rows and each
edge indexes (src_pos % 32768) within its group's slice of the table.

Per layer:
  phase A (per core, own 12544 positions):  z = dinv * (h @ W)      (TensorE)
  AllGather z -> full table [P_TOT, 128] bf16 (row-padded to 256B)
  phase B: per-edge messages are pulled with dma_gather (256B rows), a one-hot
  scatter matrix P[edge, dst_slot] is built on VectorE (is_equal vs iota), and
  TensorE computes psum[dst_slot, feat] += P.T @ msg, accumulating all edges of
  a 128-position dst tile.  Bias enters as a rank-1 matmul (binv x b), and the
  flush applies  h = relu(dinv * psum)  in one fused tensor_scalar.

Pooling: psum[feat, graph] += h_tile.T @ S_tile (S = one-hot batch matrix),
AllReduce, then the 2-layer MLP head runs replicated on every core.
"""

import math

import numpy as np
import ml_dtypes

import concourse.bass as bass
import concourse.mybir as mybir
import concourse.tile as tile
from concourse import bacc, library_config
from concourse.bass_utils import run_bass_kernel_spmd

F32 = mybir.dt.float32
BF16 = mybir.dt.bfloat16
I16 = mybir.dt.int16
OP = mybir.AluOpType


class Cfg:
    def __init__(self, N, E, IN=128, H=64, OUT=2, G=64, TPC=98, GRP=32768):
        self.N, self.E, self.IN, self.H, self.OUT, self.G = N, E, IN, H, OUT, G
        self.NCORES = 8
        self.TPC = TPC                      # dst tiles (of 128 positions) per core
        self.PPC = 128 * TPC                # positions per core
        self.P_TOT = self.NCORES * self.PPC
        self.GRP = GRP                      # src-group size (int16 index reach)
        self.NG = -(-self.P_TOT // GRP)     # number of src groups
        assert self.NG <= 4
        # reserved all-zero positions: last row of each group
        self.reserved = sorted({g * GRP - 1 for g in range(1, self.NG)} | {self.P_TOT - 1})
        assert N <= self.P_TOT - len(self.reserved)
        self.ZPAD = 128                     # padded row width of z table (bf16 -> 256B)
        assert self.ZPAD >= H
        # halves for PSUM bank packing (<= 56 tiles -> 7 banks)
        nh = -(-TPC // 56)
        base = TPC // nh
        rem = TPC % nh
        self.halves = []
        t0 = 0
        for i in range(nh):
            n = base + (1 if i < rem else 0)
            self.halves.append((t0, t0 + n))
            t0 += n
        self.CALL_CC = 32                   # gather chunks (of 128 idxs) per dma_gather
        self.PW = 16                        # one-hot build window (chunks per DVE op)
        self.NQ = 4                         # SWDGE queues (parallel desc rings)
        self.MBUFS = 4                      # message-tile buffers in flight


REAL = Cfg(N=100000, E=1600000)


def _pos_map(cfg):
    """node id -> position, skipping reserved rows."""
    pos = np.arange(cfg.N, dtype=np.int64)
    for r in cfg.reserved:
        pos += pos >= r
    return pos


def prepare(x, edge_index, batch, cfg):
    """Host-side sharding/prep. Returns (sched, per_core_inputs: list[dict])."""
    N, E, G = cfg.N, cfg.E, cfg.G
    NC, PPC, TPC, NG, GRP = cfg.NCORES, cfg.PPC, cfg.TPC, cfg.NG, cfg.GRP

    x = np.asarray(x, dtype=np.float32)
    edge_index = np.asarray(edge_index, dtype=np.int64)
    batch = np.asarray(batch, dtype=np.int64)

    pos = _pos_map(cfg)
    # Self-loops are NOT materialized as edges: each dst tile's self message is
    # its own zh tile in SBUF, added via one identity matmul per tile.  They
    # still count toward the degree.
    src = pos[edge_index[0]]
    dst = pos[edge_index[1]]

    deg = np.bincount(dst, minlength=cfg.P_TOT).astype(np.float64)
    np.add.at(deg, pos, 1.0)
    dinv = np.where(deg > 0, 1.0 / np.sqrt(np.maximum(deg, 1)), 0.0).astype(np.float32)
    binv = np.where(deg > 0, np.sqrt(deg), 0.0).astype(np.float32)

    # --- per-core edge partition (by dst) and (tile, group) run structure ---
    core_of = dst // PPC
    t_loc = (dst % PPC) >> 7
    g_of = src // GRP
    dst_loc = dst & 127
    src_loc = (src - g_of * GRP).astype(np.int64)

    counts = np.zeros((NC, TPC, NG), dtype=np.int64)
    per_core = []
    for c in range(NC):
        m = core_of == c
        key = t_loc[m] * NG + g_of[m]
        counts[c] = np.bincount(key, minlength=TPC * NG).reshape(TPC, NG)
        per_core.append((key, src_loc[m], dst_loc[m]))

    nch = -(-counts.max(axis=0) // 128)     # [TPC, NG] chunks (128 idx) per run

    # --- global chunk schedule: for half, for g, for t in half ---
    chunk_sched = {}                        # (h,g) -> list of (t, k)
    run_chunk_start = np.zeros((TPC, NG), dtype=np.int64)
    cid = 0
    for h, (ta, tb) in enumerate(cfg.halves):
        for g in range(NG):
            lst = []
            for t in range(ta, tb):
                run_chunk_start[t, g] = cid
                for k in range(nch[t, g]):
                    lst.append((t, k))
                    cid += 1
            chunk_sched[(h, g)] = lst
    NCHUNK = cid
    NIDX = NCHUNK * 128

    # last (nonempty) chunk of each tile -> stop flag
    stop_chunk = np.full(TPC, -1, dtype=np.int64)
    for t in range(TPC):
        gs = [g for g in range(NG) if nch[t, g] > 0]
        if gs:
            stop_chunk[t] = run_chunk_start[t, gs[-1]] + nch[t, gs[-1]] - 1

    # call plan: (h,g) -> list of (chunk_id_start, n_chunks)
    calls = {}
    for h, (ta, tb) in enumerate(cfg.halves):
        for g in range(NG):
            n = len(chunk_sched[(h, g)])
            if n == 0:
                calls[(h, g)] = []
                continue
            c0 = run_chunk_start[ta, g]
            calls[(h, g)] = [(c0 + a, min(cfg.CALL_CC, n - a)) for a in range(0, n, cfg.CALL_CC)]

    # group dummy index (points at a reserved zero row, local to the group)
    g_hi = [min((g + 1) * GRP, cfg.P_TOT) for g in range(NG)]
    dummy = [g_hi[g] - 1 - g * GRP for g in range(NG)]

    # --- per-core flattened idx / dst tables in global chunk order ---
    batch_pos = np.full(cfg.P_TOT, -1, dtype=np.int64)
    batch_pos[pos] = batch
    cnt = np.bincount(batch, minlength=G).astype(np.float64)
    cntinv = (1.0 / np.maximum(cnt, 1.0)).astype(np.float32)

    x_pos = np.zeros((cfg.P_TOT, cfg.IN), dtype=np.float32)
    x_pos[pos] = x

    per_core_inputs = []
    for c in range(NC):
        key, sl, dl = per_core[c]
        order = np.argsort(key, kind="stable")
        key_s, sl_s, dl_s = key[order], sl[order], dl[order]
        cnts = np.bincount(key_s, minlength=TPC * NG).reshape(TPC, NG)
        run_start_edge = np.zeros(TPC * NG, dtype=np.int64)
        np.cumsum(cnts.reshape(-1)[:-1], out=run_start_edge[1:])
        rank = np.arange(len(key_s)) - run_start_edge[key_s]
        slot = run_chunk_start.reshape(-1)[key_s] * 128 + rank

        idx_arr = np.zeros(NIDX, dtype=np.int16)
        # fill dummies per (half, group) chunk region first
        for h in range(len(cfg.halves)):
            for g in range(NG):
                lst = chunk_sched[(h, g)]
                if lst:
                    a = run_chunk_start[lst[0][0], g] * 128
                    b = a + len(lst) * 128
                    idx_arr[a:b] = dummy[g]
        idx_arr[slot] = sl_s.astype(np.int16)
        dst_arr = np.zeros(NIDX, dtype=np.float32)
        dst_arr[slot] = dl_s
        # wrapped int16 idx layout [128, NIDX/16]
        idx_tab = np.tile(idx_arr.reshape(-1, 16).T, (8, 1)).astype(np.int16)
        dst_tab = dst_arr.reshape(NCHUNK, 128).T.astype(ml_dtypes.bfloat16)

        base = c * PPC
        xT = np.ascontiguousarray(x_pos[base:base + PPC].T).astype(ml_dtypes.bfloat16)
        dinv_tab = np.ascontiguousarray(dinv[base:base + PPC].reshape(TPC, 128).T)
        # per-bank bias lhsT: binv8[s, gb*128 + p] = binv at tile (bank, slice s)
        NBANK = sum(-(-(tb - ta) // 8) for (ta, tb) in cfg.halves)
        binv8 = np.zeros((8, NBANK * 128), dtype=np.float32)
        gb = 0
        for (ta, tb) in cfg.halves:
            for b in range(-(-(tb - ta) // 8)):
                for s in range(8):
                    t = ta + b * 8 + s
                    if t < tb:
                        binv8[s, gb * 128:(gb + 1) * 128] = \
                            binv[base + t * 128: base + (t + 1) * 128]
                gb += 1

        bp = batch_pos[base:base + PPC].reshape(TPC, 128)
        S = np.zeros((128, TPC, G), dtype=ml_dtypes.bfloat16)
        for t in range(TPC):
            valid = bp[t] >= 0
            S[valid, t, bp[t][valid]] = 1.0

        per_core_inputs.append(dict(
            xT=xT, idx=idx_tab, dstl=dst_tab,
            dinv=dinv_tab.astype(np.float32), binv8=binv8,
            S=S.reshape(128, TPC * G),
            cnti=np.broadcast_to(cntinv[None, :], (cfg.H, G)).copy().astype(np.float32),
        ))

    iota = np.broadcast_to(np.arange(128, dtype=np.float32)[None, None, :],
                           (128, cfg.PW, 128)).astype(ml_dtypes.bfloat16).copy()
    ident = np.eye(128, dtype=np.float32).astype(ml_dtypes.bfloat16)
    for d in per_core_inputs:
        d["iota"] = iota
        d["ident"] = ident

    # per-bank last chunk (emission order = ascending cid) -> carries stop flag
    NBANK = sum(-(-(tb - ta) // 8) for (ta, tb) in cfg.halves)
    bank_of_tile = np.zeros(TPC, dtype=np.int64)
    bank_stop = np.full(NBANK, -1, dtype=np.int64)
    gb = 0
    for (ta, tb) in cfg.halves:
        for b in range(-(-(tb - ta) // 8)):
            for s in range(8):
                t = ta + b * 8 + s
                if t < tb:
                    bank_of_tile[t] = gb
                    bank_stop[gb] = max(bank_stop[gb], stop_chunk[t])
            gb += 1

    sched = dict(nch=nch, chunk_sched=chunk_sched, calls=calls, stop_chunk=stop_chunk,
                 run_chunk_start=run_chunk_start, NCHUNK=NCHUNK, NIDX=NIDX,
                 dummy=dummy, g_hi=g_hi, NBANK=NBANK, bank_of_tile=bank_of_tile,
                 bank_stop=bank_stop)
    return sched, per_core_inputs


def _blk_diag_b(b):
    b = np.asarray(b, np.float32)
    H = b.shape[0]
    out = np.zeros((8, 8 * H), dtype=np.float32)
    for s in range(8):
        out[s, s * H:(s + 1) * H] = b
    return out


def add_weight_inputs(per_core_inputs, W1, b1, W2, b2, W3, b3, Wm1, bm1, Wm2, bm2):
    w = dict(
        W1=np.asarray(W1, np.float32).astype(ml_dtypes.bfloat16),
        W2=np.asarray(W2, np.float32).astype(ml_dtypes.bfloat16),
        W3=np.asarray(W3, np.float32).astype(ml_dtypes.bfloat16),
        b1=_blk_diag_b(b1),
        b2=_blk_diag_b(b2),
        b3=_blk_diag_b(b3),
        Wm1=np.asarray(Wm1, np.float32),
        bm1=np.asarray(bm1, np.float32)[:, None],
        Wm2=np.asarray(Wm2, np.float32),
        bm2=np.asarray(bm2, np.float32)[:, None],
    )
    for d in per_core_inputs:
        d.update(w)


def build_program(cfg, sched):
    IN, H, OUT, G = cfg.IN, cfg.H, cfg.OUT, cfg.G
    TPC, PPC, NG = cfg.TPC, cfg.PPC, cfg.NG
    NCHUNK, NIDX = sched["NCHUNK"], sched["NIDX"]
    nch, stop_chunk = sched["nch"], sched["stop_chunk"]
    run_chunk_start = sched["run_chunk_start"]

    nc = bacc.Bacc("TRN2", target_bir_lowering=False, debug=False,
                   num_devices=cfg.NCORES,
                   num_swdge_queues=getattr(cfg, "NQ", 1))

    di = lambda n, s, d: nc.dram_tensor(n, s, d, kind="ExternalInput")
    xT_p = di("xT", [IN, PPC], BF16)
    idx_p = di("idx", [128, NIDX // 16], I16)
    dst_p = di("dstl", [128, NCHUNK], BF16)
    iota_p = di("iota", [128, cfg.PW, 128], BF16)
    dinv_p = di("dinv", [128, TPC], F32)
    binv_p = di("binv8", [8, sched["NBANK"] * 128], F32)
    S_p = di("S", [128, TPC * G], BF16)
    ident_p = di("ident", [128, 128], BF16)
    cnti_p = di("cnti", [H, G], F32)
    W_p = [di("W1", [IN, H], BF16), di("W2", [H, H], BF16), di("W3", [H, H], BF16)]
    b_p = [di("b1", [8, 8 * H], F32), di("b2", [8, 8 * H], F32), di("b3", [8, 8 * H], F32)]
    Wm1_p = di("Wm1", [H, H], F32)
    bm1_p = di("bm1", [H, 1], F32)
    Wm2_p = di("Wm2", [H, OUT], F32)
    bm2_p = di("bm2", [OUT, 1], F32)
    out_p = nc.dram_tensor("out", [OUT, G], F32, kind="ExternalOutput")
    dbg = getattr(cfg, "debug_taps", False)
    if dbg:
        zdbg_p = nc.dram_tensor("zdbg", [3, cfg.P_TOT, cfg.ZPAD], BF16,
                                kind="ExternalOutput")
        hdbg_p = nc.dram_tensor("hdbg", [3, 128, TPC, cfg.ZPAD], BF16,
                                kind="ExternalOutput")
        pdbg_p = nc.dram_tensor("pdbg", [H, G], F32, kind="ExternalOutput")

    z_full = nc.dram_tensor("z_full", [cfg.P_TOT, cfg.ZPAD], BF16, addr_space="Shared")
    pool_full = nc.dram_tensor("pool_full", [H, G], F32, addr_space="Shared")
    z_loc = nc.dram_tensor("z_loc", [PPC, cfg.ZPAD], BF16)
    h_dram = nc.dram_tensor("h_dram", [PPC, cfg.ZPAD], BF16)
    pool_loc = nc.dram_tensor("pool_loc", [H, G], F32)

    rg = [list(range(cfg.NCORES))]

    with tile.TileContext(nc) as tc:
        nc.gpsimd.load_library(library_config.mlp)
        tc.strict_bb_all_engine_barrier()

        import contextlib
        with contextlib.ExitStack() as ctx:
            cpool = ctx.enter_context(tc.tile_pool(name="consts", bufs=1))

            def load(p, shape, dt):
                t = cpool.tile(shape, dt, tag=p.name)
                nc.sync.dma_start(t[:], p[:])
                return t

            xT_sb = load(xT_p, [IN, PPC], BF16)
            idx_sb = load(idx_p, [128, NIDX // 16], I16)
            dst_sb = load(dst_p, [128, NCHUNK], BF16)
            iota_sb = load(iota_p, [128, cfg.PW, 128], BF16)
            dinv_sb = load(dinv_p, [128, TPC], F32)
            S_sb = load(S_p, [128, TPC * G], BF16)
            ident_sb = load(ident_p, [128, 128], BF16)
            cnti_sb = load(cnti_p, [H, G], F32)
            W_sb = [load(W_p[i], [IN if i == 0 else H, H], BF16) for i in range(3)]
            b_sb = [load(b_p[i], [8, 8 * H], F32) for i in range(3)]
            binv_sb = load(binv_p, [8, sched["NBANK"] * 128], F32)
            Wm1_sb = load(Wm1_p, [H, H], F32)
            bm1_sb = load(bm1_p, [H, 1], F32)
            Wm2_sb = load(Wm2_p, [H, OUT], F32)
            bm2_sb = load(bm2_p, [OUT, 1], F32)

            zh = cpool.tile([128, TPC, cfg.ZPAD], BF16, tag="zh")    # z / h staging
            hT_sb = cpool.tile([128, PPC], BF16, tag="hT")
            nc.vector.memset(zh[:], 0.0)

            for L in range(3):
                K = IN if L == 0 else H
                # ---- phase A: z = dinv * (h @ W) ----
                with tc.tile_pool(name=f"psA{L}", bufs=4, space="PSUM") as psA:
                    for t in range(TPC):
                        ps = psA.tile([128, H], F32, tag="psA")
                        lhsT = (xT_sb if L == 0 else hT_sb)[0:K, t * 128:(t + 1) * 128]
                        nc.tensor.matmul(ps[:], lhsT, W_sb[L][:], start=True, stop=True)
                        nc.vector.tensor_scalar(
                            zh[:, t, 0:H], ps[:], dinv_sb[:, t:t + 1], None, OP.mult)
                nc.sync.dma_start(
                    out=z_loc.rearrange("(t p) c -> p t c", p=128), in_=zh[:])
                tc.strict_bb_all_engine_barrier()
                nc.gpsimd.collective_compute(
                    "AllGather", OP.bypass, replica_groups=rg,
                    ins=[z_loc[:]], outs=[z_full[:]])
                tc.strict_bb_all_engine_barrier()
                if dbg:
                    nc.sync.dma_start(out=zdbg_p[L], in_=z_full[:])

                # ---- phase B: aggregate ----
                with contextlib.ExitStack() as bctx:
                    psB = bctx.enter_context(
                        tc.tile_pool(name=f"psB{L}", bufs=8, space="PSUM"))
                    mpool = bctx.enter_context(
                        tc.tile_pool(name=f"msg{L}", bufs=getattr(cfg, "MBUFS", 3)))
                    ppool = bctx.enter_context(
                        tc.tile_pool(name=f"P{L}", bufs=3))
                    bank_base = 0
                    for h, (ta, tb) in enumerate(cfg.halves):
                        if h > 0:
                            bank_base += -(-(cfg.halves[h-1][1] - cfg.halves[h-1][0]) // 8)
                        nbank = -(-(tb - ta) // 8)
                        banks = [psB.tile([128, 512], F32, tag="bank",
                                          name=f"bank_L{L}_h{h}_{i}")
                                 for i in range(nbank)]

                        def bank_ap(t):
                            tl = t - ta
                            return banks[tl // 8][:, (tl % 8) * H:(tl % 8) * H + H]

                        for b in range(nbank):
                            gb = bank_base + b
                            nc.tensor.matmul(
                                banks[b][:, 0:8 * H],
                                binv_sb[:, gb * 128:(gb + 1) * 128],
                                b_sb[L][:], start=True, stop=False)
                        # self-loop contribution: += I @ zh[:, t, :]
                        for t in range(ta, tb):
                            gb = sched["bank_of_tile"][t]
                            last_of_bank = (t == tb - 1) or \
                                (sched["bank_of_tile"][t + 1] != gb)
                            nc.tensor.matmul(
                                bank_ap(t), ident_sb[:],
                                zh[:, t, 0:H], start=False,
                                stop=(last_of_bank and sched["bank_stop"][gb] < 0))
                        qn = 0
                        for g in range(NG):
                            glo = g * cfg.GRP
                            z_src = z_full[glo:sched["g_hi"][g], :]
                            for (c0, ncc) in sched["calls"][(h, g)]:
                                msg = mpool.tile([128, cfg.CALL_CC, cfg.ZPAD], BF16, tag="msg")
                                nidx = ncc * 128
                                if getattr(cfg, "ablate", None) != "no_gather":
                                    nc.gpsimd.dma_gather(
                                        msg[:, 0:ncc, :], z_src,
                                        idx_sb[:, c0 * 8:(c0 + ncc) * 8],
                                        nidx, nidx, cfg.ZPAD,
                                        single_packet=False,
                                        queue_num=qn % getattr(cfg, "NQ", 1))
                                    qn += 1
                                if getattr(cfg, "ablate", None) == "gather_only":
                                    continue
                                for w0 in range(0, ncc, cfg.PW):
                                    w = min(cfg.PW, ncc - w0)
                                    P = ppool.tile([128, cfg.PW, 128], BF16, tag="P")
                                    nc.vector.tensor_tensor(
                                        P[:, 0:w, :], iota_sb[:, 0:w, :],
                                        dst_sb[:, c0 + w0:c0 + w0 + w, None]
                                        .to_broadcast((128, w, 128)),
                                        OP.is_equal)
                                    for j in range(w):
                                        cid = c0 + w0 + j
                                        t, k = sched["chunk_sched"][(h, g)][cid - run_chunk_start[ta, g]]
                                        gb = sched["bank_of_tile"][t]
                                        nc.tensor.matmul(
                                            bank_ap(t), P[:, j, :],
                                            msg[:, w0 + j, 0:H],
                                            start=False,
                                            stop=(cid == sched["bank_stop"][gb]))
                        for t in range(ta, tb):
                            nc.vector.tensor_scalar(
                                zh[:, t, 0:H], bank_ap(t),
                                dinv_sb[:, t:t + 1], 0.0, OP.mult, OP.max)

                if dbg:
                    nc.sync.dma_start(out=hdbg_p[L], in_=zh[:])
                if L < 2:
                    nc.sync.dma_start(
                        out=h_dram.rearrange("(t p) c -> p t c", p=128), in_=zh[:])
                    tc.strict_bb_all_engine_barrier()
                    nc.sync.dma_start_transpose(out=hT_sb[:], in_=h_dram[:])

            # ---- mean pooling ----
            with tc.tile_pool(name="psP", bufs=2, space="PSUM") as psP:
                pp = psP.tile([H, G], F32, tag="pp")
                for t in range(TPC):
                    nc.tensor.matmul(
                        pp[:], zh[:, t, 0:H], S_sb[:, t * G:(t + 1) * G],
                        start=(t == 0), stop=(t == TPC - 1))
                pool_sb = cpool.tile([H, G], F32, tag="pool_sb")
                nc.vector.tensor_copy(pool_sb[:], pp[:])
            nc.sync.dma_start(out=pool_loc[:], in_=pool_sb[:])
            if dbg:
                nc.sync.dma_start(out=pdbg_p[:], in_=pool_sb[:])
            tc.strict_bb_all_engine_barrier()
            nc.gpsimd.collective_compute(
                "AllReduce", OP.add, replica_groups=rg,
                ins=[pool_loc[:]], outs=[pool_full[:]])
            tc.strict_bb_all_engine_barrier()

            # ---- MLP head (replicated) ----
            with tc.tile_pool(name="psM", bufs=2, space="PSUM") as psM:
                pooled = cpool.tile([H, G], F32, tag="pooled")
                nc.sync.dma_start(pooled[:], pool_full[:])
                pooln = cpool.tile([H, G], F32, tag="pooln")
                nc.vector.tensor_tensor(pooln[:], pooled[:], cnti_sb[:], OP.mult)
                ps1 = psM.tile([H, G], F32, tag="ps1")
                nc.tensor.matmul(ps1[:], Wm1_sb[:], pooln[:], start=True, stop=True)
                z1 = cpool.tile([H, G], F32, tag="z1")
                nc.scalar.activation(z1[:], ps1[:],
                                     mybir.ActivationFunctionType.Relu,
                                     bias=bm1_sb[:, 0:1], scale=1.0)
                ps2 = psM.tile([OUT, G], F32, tag="ps2")
                nc.tensor.matmul(ps2[:], Wm2_sb[:], z1[:], start=True, stop=True)
                out_sb = cpool.tile([OUT, G], F32, tag="out_sb")
                nc.vector.tensor_scalar(out_sb[:], ps2[:], bm2_sb[:, 0:1], None, OP.add)
            nc.sync.dma_start(out=out_p[:], in_=out_sb[:])

    nc.compile()
    return nc


def run(inputs, cfg=REAL, via="hw", trace=False):
    """inputs: the reference.setup_inputs() dict (numpy). Returns [G, OUT] fp32."""
    sched, pci = prepare(inputs["x"], inputs["edge_index"], inputs["batch"], cfg)
    add_weight_inputs(pci, inputs["W1"], inputs["b1"], inputs["W2"], inputs["b2"],
                      inputs["W3"], inputs["b3"], inputs["Wm1"], inputs["bm1"],
                      inputs["Wm2"], inputs["bm2"])
    nc = build_program(cfg, sched)
    if via == "sim":
        from concourse.bass_interp import MultiCoreSim
        sim = MultiCoreSim(nc, num_cores=cfg.NCORES, require_finite=False,
                           require_nnan=False)
        for c, core in sim.cores.items():
            for k, v in pci[c].items():
                core.tensor(k)[:] = v
        sim.simulate(check_with_hw=False)
        out = np.array(sim.cores[0].tensor("out"))
        taps = None
        if getattr(cfg, "debug_taps", False):
            taps = {k: {c: np.array(sim.cores[c].tensor(k)) for c in sim.cores}
                    for k in ("zdbg", "hdbg", "pdbg")}
        return out.T.copy(), taps
    br = run_bass_kernel_spmd(nc, pci, list(range(cfg.NCORES)), trace=trace)
    out = br.results[0]["out"]
    return np.asarray(out, np.float32).T.copy(), br


def kernel(**inputs):
    inputs = {k: np.asarray(v) for k, v in inputs.items()}
    out, _ = run(inputs, REAL, via="hw")
    return out


# revision 35
# speedup vs baseline: 12.9273x; 2.7860x over previous
"""AfterShockGNN (3-layer GCN + mean-pool + MLP head) on 8 Trainium2 NeuronCores.

Strategy
--------
Nodes are relabeled into a padded "position" space of NCORES*PPC rows with a few
reserved all-zero rows placed so that every gather index fits in int16:
the pre-aggregation feature table z is split into GROUPS of 32768 rows and each
edge indexes (src_pos % 32768) within its group's slice of the table.

Per layer:
  phase A (per core, own 12544 positions):  z = dinv * (h @ W)      (TensorE)
  AllGather z -> full table [P_TOT, 128] bf16 (row-padded to 256B)
  phase B: per-edge messages are pulled with dma_gather (256B rows), a one-hot
  scatter matrix P[edge, dst_slot] is built on VectorE (is_equal vs iota), and
  TensorE computes psum[dst_slot, feat] += P.T @ msg, accumulating all edges of
  a 128-position dst tile.  Bias enters as a rank-1 matmul (binv x b), and the
  flush applies  h = relu(dinv * psum)  in one fused tensor_scalar.

Pooling: psum[feat, graph] += h_tile.T @ S_tile (S = one-hot batch matrix),
AllReduce, then the 2-layer MLP head runs replicated on every core.
"""

import math

import numpy as np
import ml_dtypes

import concourse.bass as bass
import concourse.mybir as mybir
import concourse.tile as tile
from concourse import bacc, library_config
from concourse.bass_utils import run_bass_kernel_spmd

F32 = mybir.dt.float32
BF16 = mybir.dt.bfloat16
I16 = mybir.dt.int16
OP = mybir.AluOpType


class Cfg:
    def __init__(self, N, E, IN=128, H=64, OUT=2, G=64, TPC=98, GRP=32768):
        self.N, self.E, self.IN, self.H, self.OUT, self.G = N, E, IN, H, OUT, G
        self.NCORES = 8
        self.TPC = TPC                      # dst tiles (of 128 positions) per core
        self.PPC = 128 * TPC                # positions per core
        self.P_TOT = self.NCORES * self.PPC
        self.GRP = GRP                      # src-group size (int16 index reach)
        self.NG = -(-self.P_TOT // GRP)     # number of src groups
        assert self.NG <= 4
        # reserved all-zero positions: last row of each group
        self.reserved = sorted({g * GRP - 1 for g in range(1, self.NG)} | {self.P_TOT - 1})
        assert N <= self.P_TOT - len(self.reserved)
        self.ZPAD = 128                     # padded row width of z table (bf16 -> 256B)
        assert self.ZPAD >= H
        # halves for PSUM bank packing (<= 56 tiles -> 7 banks)
        nh = -(-TPC // 56)
        base = TPC // nh
        rem = TPC % nh
        self.halves = []
        t0 = 0
        for i in range(nh):
            n = base + (1 if i < rem else 0)
            self.halves.append((t0, t0 + n))
            t0 += n
        self.CALL_CC = 32                   # gather chunks (of 128 idxs) per dma_gather
        self.PW = 16                        # one-hot build window (chunks per DVE op)
        self.NQ = 4                         # SWDGE queues (parallel desc rings)
        self.MBUFS = 4                      # message-tile buffers in flight


REAL = Cfg(N=100000, E=1600000)


def _pos_map(cfg):
    """node id -> position, skipping reserved rows."""
    pos = np.arange(cfg.N, dtype=np.int64)
    for r in cfg.reserved:
        pos += pos >= r
    return pos


def prepare(x, edge_index, batch, cfg):
    """Host-side sharding/prep. Returns (sched, per_core_inputs: list[dict])."""
    N, E, G = cfg.N, cfg.E, cfg.G
    NC, PPC, TPC, NG, GRP = cfg.NCORES, cfg.PPC, cfg.TPC, cfg.NG, cfg.GRP

    x = np.asarray(x, dtype=np.float32)
    edge_index = np.asarray(edge_index, dtype=np.int64)
    batch = np.asarray(batch, dtype=np.int64)

    pos = _pos_map(cfg)
    # Self-loops are NOT materialized as edges: each dst tile's self message is
    # its own zh tile in SBUF, added via one identity matmul per tile.  They
    # still count toward the degree.
    src = pos[edge_index[0]]
    dst = pos[edge_index[1]]

    deg = np.bincount(dst, minlength=cfg.P_TOT).astype(np.float64)
    np.add.at(deg, pos, 1.0)
    dinv = np.where(deg > 0, 1.0 / np.sqrt(np.maximum(deg, 1)), 0.0).astype(np.float32)
    binv = np.where(deg > 0, np.sqrt(deg), 0.0).astype(np.float32)

    # --- per-core edge partition (by dst) and (tile, group) run structure ---
    core_of = dst // PPC
    t_loc = (dst % PPC) >> 7
    g_of = src // GRP
    dst_loc = dst & 127
    src_loc = (src - g_of * GRP).astype(np.int64)

    counts = np.zeros((NC, TPC, NG), dtype=np.int64)
    per_core = []
    for c in range(NC):
        m = core_of == c
        key = t_loc[m] * NG + g_of[m]
        counts[c] = np.bincount(key, minlength=TPC * NG).reshape(TPC, NG)
        per_core.append((key, src_loc[m], dst_loc[m]))

    nch = -(-counts.max(axis=0) // 128)     # [TPC, NG] chunks (128 idx) per run

    # --- global chunk schedule: for half, for g, for t in half ---
    chunk_sched = {}                        # (h,g) -> list of (t, k)
    run_chunk_start = np.zeros((TPC, NG), dtype=np.int64)
    cid = 0
    for h, (ta, tb) in enumerate(cfg.halves):
        for g in range(NG):
            lst = []
            for t in range(ta, tb):
                run_chunk_start[t, g] = cid
                for k in range(nch[t, g]):
                    lst.append((t, k))
                    cid += 1
            chunk_sched[(h, g)] = lst
    NCHUNK = cid
    NIDX = NCHUNK * 128

    # last (nonempty) chunk of each tile -> stop flag
    stop_chunk = np.full(TPC, -1, dtype=np.int64)
    for t in range(TPC):
        gs = [g for g in range(NG) if nch[t, g] > 0]
        if gs:
            stop_chunk[t] = run_chunk_start[t, gs[-1]] + nch[t, gs[-1]] - 1

    # call plan: (h,g) -> list of (chunk_id_start, n_chunks)
    calls = {}
    for h, (ta, tb) in enumerate(cfg.halves):
        for g in range(NG):
            n = len(chunk_sched[(h, g)])
            if n == 0:
                calls[(h, g)] = []
                continue
            c0 = run_chunk_start[ta, g]
            calls[(h, g)] = [(c0 + a, min(cfg.CALL_CC, n - a)) for a in range(0, n, cfg.CALL_CC)]

    # group dummy index (points at a reserved zero row, local to the group)
    g_hi = [min((g + 1) * GRP, cfg.P_TOT) for g in range(NG)]
    dummy = [g_hi[g] - 1 - g * GRP for g in range(NG)]

    # --- per-core flattened idx / dst tables in global chunk order ---
    batch_pos = np.full(cfg.P_TOT, -1, dtype=np.int64)
    batch_pos[pos] = batch
    cnt = np.bincount(batch, minlength=G).astype(np.float64)
    cntinv = (1.0 / np.maximum(cnt, 1.0)).astype(np.float32)

    x_pos = np.zeros((cfg.P_TOT, cfg.IN), dtype=np.float32)
    x_pos[pos] = x

    per_core_inputs = []
    for c in range(NC):
        key, sl, dl = per_core[c]
        order = np.argsort(key, kind="stable")
        key_s, sl_s, dl_s = key[order], sl[order], dl[order]
        cnts = np.bincount(key_s, minlength=TPC * NG).reshape(TPC, NG)
        run_start_edge = np.zeros(TPC * NG, dtype=np.int64)
        np.cumsum(cnts.reshape(-1)[:-1], out=run_start_edge[1:])
        rank = np.arange(len(key_s)) - run_start_edge[key_s]
        slot = run_chunk_start.reshape(-1)[key_s] * 128 + rank

        idx_arr = np.zeros(NIDX, dtype=np.int16)
        # fill dummies per (half, group) chunk region first
        for h in range(len(cfg.halves)):
            for g in range(NG):
                lst = chunk_sched[(h, g)]
                if lst:
                    a = run_chunk_start[lst[0][0], g] * 128
                    b = a + len(lst) * 128
                    idx_arr[a:b] = dummy[g]
        idx_arr[slot] = sl_s.astype(np.int16)
        dst_arr = np.zeros(NIDX, dtype=np.float32)
        dst_arr[slot] = dl_s
        # wrapped int16 idx layout [128, NIDX/16]
        idx_tab = np.tile(idx_arr.reshape(-1, 16).T, (8, 1)).astype(np.int16)
        dst_tab = dst_arr.reshape(NCHUNK, 128).T.astype(ml_dtypes.bfloat16)

        base = c * PPC
        xT = np.ascontiguousarray(x_pos[base:base + PPC].T).astype(ml_dtypes.bfloat16)
        dinv_tab = np.ascontiguousarray(dinv[base:base + PPC].reshape(TPC, 128).T)
        # per-bank bias lhsT: binv8[s, gb*128 + p] = binv at tile (bank, slice s)
        NBANK = sum(-(-(tb - ta) // 8) for (ta, tb) in cfg.halves)
        binv8 = np.zeros((8, NBANK * 128), dtype=np.float32)
        gb = 0
        for (ta, tb) in cfg.halves:
            for b in range(-(-(tb - ta) // 8)):
                for s in range(8):
                    t = ta + b * 8 + s
                    if t < tb:
                        binv8[s, gb * 128:(gb + 1) * 128] = \
                            binv[base + t * 128: base + (t + 1) * 128]
                gb += 1

        bp = batch_pos[base:base + PPC].reshape(TPC, 128)
        S = np.zeros((128, TPC, G), dtype=ml_dtypes.bfloat16)
        for t in range(TPC):
            valid = bp[t] >= 0
            S[valid, t, bp[t][valid]] = 1.0

        per_core_inputs.append(dict(
            xT=xT, idx=idx_tab, dstl=dst_tab,
            dinv=dinv_tab.astype(np.float32), binv8=binv8,
            S=S.reshape(128, TPC * G),
            cnti=np.broadcast_to(cntinv[None, :], (cfg.H, G)).copy().astype(np.float32),
        ))

    iota = np.broadcast_to(np.arange(128, dtype=np.float32)[None, None, :],
                           (128, cfg.PW, 128)).astype(ml_dtypes.bfloat16).copy()
    ident = np.eye(128, dtype=np.float32).astype(ml_dtypes.bfloat16)
    for d in per_core_inputs:
        d["iota"] = iota
        d["ident"] = ident

    # per-bank last chunk (emission order = ascending cid) -> carries stop flag
    NBANK = sum(-(-(tb - ta) // 8) for (ta, tb) in cfg.halves)
    bank_of_tile = np.zeros(TPC, dtype=np.int64)
    bank_stop = np.full(NBANK, -1, dtype=np.int64)
    gb = 0
    for (ta, tb) in cfg.halves:
        for b in range(-(-(tb - ta) // 8)):
            for s in range(8):
                t = ta + b * 8 + s
                if t < tb:
                    bank_of_tile[t] = gb
                    bank_stop[gb] = max(bank_stop[gb], stop_chunk[t])
            gb += 1

    sched = dict(nch=nch, chunk_sched=chunk_sched, calls=calls, stop_chunk=stop_chunk,
                 run_chunk_start=run_chunk_start, NCHUNK=NCHUNK, NIDX=NIDX,
                 dummy=dummy, g_hi=g_hi, NBANK=NBANK, bank_of_tile=bank_of_tile,
                 bank_stop=bank_stop)
    return sched, per_core_inputs


def _blk_diag_b(b):
    b = np.asarray(b, np.float32)
    H = b.shape[0]
    out = np.zeros((8, 8 * H), dtype=np.float32)
    for s in range(8):
        out[s, s * H:(s + 1) * H] = b
    return out


def add_weight_inputs(per_core_inputs, W1, b1, W2, b2, W3, b3, Wm1, bm1, Wm2, bm2):
    w = dict(
        W1=np.asarray(W1, np.float32).astype(ml_dtypes.bfloat16),
        W2=np.asarray(W2, np.float32).astype(ml_dtypes.bfloat16),
        W3=np.asarray(W3, np.float32).astype(ml_dtypes.bfloat16),
        b1=_blk_diag_b(b1),
        b2=_blk_diag_b(b2),
        b3=_blk_diag_b(b3),
        Wm1=np.asarray(Wm1, np.float32),
        bm1=np.asarray(bm1, np.float32)[:, None],
        Wm2=np.asarray(Wm2, np.float32),
        bm2=np.asarray(bm2, np.float32)[:, None],
    )
    for d in per_core_inputs:
        d.update(w)


def build_program(cfg, sched):
    IN, H, OUT, G = cfg.IN, cfg.H, cfg.OUT, cfg.G
    TPC, PPC, NG = cfg.TPC, cfg.PPC, cfg.NG
    NCHUNK, NIDX = sched["NCHUNK"], sched["NIDX"]
    nch, stop_chunk = sched["nch"], sched["stop_chunk"]
    run_chunk_start = sched["run_chunk_start"]

    nc = bacc.Bacc("TRN2", target_bir_lowering=False, debug=False,
                   num_devices=cfg.NCORES,
                   num_swdge_queues=getattr(cfg, "NQ", 1))

    di = lambda n, s, d: nc.dram_tensor(n, s, d, kind="ExternalInput")
    xT_p = di("xT", [IN, PPC], BF16)
    idx_p = di("idx", [128, NIDX // 16], I16)
    dst_p = di("dstl", [128, NCHUNK], BF16)
    iota_p = di("iota", [128, cfg.PW, 128], BF16)
    dinv_p = di("dinv", [128, TPC], F32)
    binv_p = di("binv8", [8, sched["NBANK"] * 128], F32)
    S_p = di("S", [128, TPC * G], BF16)
    ident_p = di("ident", [128, 128], BF16)
    cnti_p = di("cnti", [H, G], F32)
    W_p = [di("W1", [IN, H], BF16), di("W2", [H, H], BF16), di("W3", [H, H], BF16)]
    b_p = [di("b1", [8, 8 * H], F32), di("b2", [8, 8 * H], F32), di("b3", [8, 8 * H], F32)]
    Wm1_p = di("Wm1", [H, H], F32)
    bm1_p = di("bm1", [H, 1], F32)
    Wm2_p = di("Wm2", [H, OUT], F32)
    bm2_p = di("bm2", [OUT, 1], F32)
    out_p = nc.dram_tensor("out", [OUT, G], F32, kind="ExternalOutput")
    dbg = getattr(cfg, "debug_taps", False)
    if dbg:
        zdbg_p = nc.dram_tensor("zdbg", [3, cfg.P_TOT, cfg.ZPAD], BF16,
                                kind="ExternalOutput")
        hdbg_p = nc.dram_tensor("hdbg", [3, 128, TPC, cfg.ZPAD], BF16,
                                kind="ExternalOutput")
        pdbg_p = nc.dram_tensor("pdbg", [H, G], F32, kind="ExternalOutput")

    z_full = nc.dram_tensor("z_full", [cfg.P_TOT, cfg.ZPAD], BF16, addr_space="Shared")
    pool_full = nc.dram_tensor("pool_full", [H, G], F32, addr_space="Shared")
    z_loc = nc.dram_tensor("z_loc", [PPC, cfg.ZPAD], BF16)
    h_dram = nc.dram_tensor("h_dram", [PPC, cfg.ZPAD], BF16)
    pool_loc = nc.dram_tensor("pool_loc", [H, G], F32)

    rg = [list(range(cfg.NCORES))]

    with tile.TileContext(nc) as tc:
        nc.gpsimd.load_library(library_config.mlp)
        tc.strict_bb_all_engine_barrier()

        import contextlib
        with contextlib.ExitStack() as ctx:
            cpool = ctx.enter_context(tc.tile_pool(name="consts", bufs=1))

            def load(p, shape, dt):
                t = cpool.tile(shape, dt, tag=p.name)
                nc.sync.dma_start(t[:], p[:])
                return t

            xT_sb = load(xT_p, [IN, PPC], BF16)
            idx_sb = load(idx_p, [128, NIDX // 16], I16)
            dst_sb = load(dst_p, [128, NCHUNK], BF16)
            iota_sb = load(iota_p, [128, cfg.PW, 128], BF16)
            dinv_sb = load(dinv_p, [128, TPC], F32)
            S_sb = load(S_p, [128, TPC * G], BF16)
            ident_sb = load(ident_p, [128, 128], BF16)
            cnti_sb = load(cnti_p, [H, G], F32)
            W_sb = [load(W_p[i], [IN if i == 0 else H, H], BF16) for i in range(3)]
            b_sb = [load(b_p[i], [8, 8 * H], F32) for i in range(3)]
            binv_sb = load(binv_p, [8, sched["NBANK"] * 128], F32)
            Wm1_sb = load(Wm1_p, [H, H], F32)
            bm1_sb = load(bm1_p, [H, 1], F32)
            Wm2_sb = load(Wm2_p, [H, OUT], F32)
            bm2_sb = load(bm2_p, [OUT, 1], F32)

            zh = cpool.tile([128, TPC, cfg.ZPAD], BF16, tag="zh")    # z / h staging
            hT_sb = cpool.tile([128, PPC], BF16, tag="hT")
            nc.vector.memset(zh[:], 0.0)

            for L in range(3):
                K = IN if L == 0 else H
                # ---- phase A: z = dinv * (h @ W) ----
                with tc.tile_pool(name=f"psA{L}", bufs=4, space="PSUM") as psA:
                    for t in range(TPC):
                        ps = psA.tile([128, H], F32, tag="psA")
                        lhsT = (xT_sb if L == 0 else hT_sb)[0:K, t * 128:(t + 1) * 128]
                        nc.tensor.matmul(ps[:], lhsT, W_sb[L][:], start=True, stop=True)
                        nc.vector.tensor_scalar(
                            zh[:, t, 0:H], ps[:], dinv_sb[:, t:t + 1], None, OP.mult)
                nc.sync.dma_start(
                    out=z_loc.rearrange("(t p) c -> p t c", p=128), in_=zh[:])
                tc.strict_bb_all_engine_barrier()
                nc.gpsimd.collective_compute(
                    "AllGather", OP.bypass, replica_groups=rg,
                    ins=[z_loc[:]], outs=[z_full[:]])
                tc.strict_bb_all_engine_barrier()
                if dbg:
                    nc.sync.dma_start(out=zdbg_p[L], in_=z_full[:])

                # ---- phase B: aggregate ----
                with contextlib.ExitStack() as bctx:
                    psB = bctx.enter_context(
                        tc.tile_pool(name=f"psB{L}", bufs=8, space="PSUM"))
                    mpool = bctx.enter_context(
                        tc.tile_pool(name=f"msg{L}", bufs=getattr(cfg, "MBUFS", 3)))
                    ppool = bctx.enter_context(
                        tc.tile_pool(name=f"P{L}", bufs=3))
                    bank_base = 0
                    for h, (ta, tb) in enumerate(cfg.halves):
                        if h > 0:
                            bank_base += -(-(cfg.halves[h-1][1] - cfg.halves[h-1][0]) // 8)
                        nbank = -(-(tb - ta) // 8)
                        banks = [psB.tile([128, 512], F32, tag="bank",
                                          name=f"bank_L{L}_h{h}_{i}")
                                 for i in range(nbank)]

                        def bank_ap(t):
                            tl = t - ta
                            return banks[tl // 8][:, (tl % 8) * H:(tl % 8) * H + H]

                        for b in range(nbank):
                            gb = bank_base + b
                            nc.tensor.matmul(
                                banks[b][:, 0:8 * H],
                                binv_sb[:, gb * 128:(gb + 1) * 128],
                                b_sb[L][:], start=True, stop=False)
                        # self-loop contribution: += I @ zh[:, t, :]
                        for t in range(ta, tb):
                            gb = sched["bank_of_tile"][t]
                            last_of_bank = (t == tb - 1) or \
                                (sched["bank_of_tile"][t + 1] != gb)
                            nc.tensor.matmul(
                                bank_ap(t), ident_sb[:],
                                zh[:, t, 0:H], start=False,
                                stop=(last_of_bank and sched["bank_stop"][gb] < 0))
                        qn = 0
                        for g in range(NG):
                            glo = g * cfg.GRP
                            z_src = z_full[glo:sched["g_hi"][g], :]
                            for (c0, ncc) in sched["calls"][(h, g)]:
                                msg = mpool.tile([128, cfg.CALL_CC, cfg.ZPAD], BF16, tag="msg")
                                nidx = ncc * 128
                                if getattr(cfg, "ablate", None) != "no_gather":
                                    nc.gpsimd.dma_gather(
                                        msg[:, 0:ncc, :], z_src,
                                        idx_sb[:, c0 * 8:(c0 + ncc) * 8],
                                        nidx, nidx, cfg.ZPAD,
                                        single_packet=False,
                                        queue_num=qn % getattr(cfg, "NQ", 1))
                                    qn += 1
                                if getattr(cfg, "ablate", None) == "gather_only":
                                    continue
                                for w0 in range(0, ncc, cfg.PW):
                                    w = min(cfg.PW, ncc - w0)
                                    P = ppool.tile([128, cfg.PW, 128], BF16, tag="P")
                                    nc.vector.tensor_tensor(
                                        P[:, 0:w, :], iota_sb[:, 0:w, :],
                                        dst_sb[:, c0 + w0:c0 + w0 + w, None]
                                        .to_broadcast((128, w, 128)),
                                        OP.is_equal)
                                    for j in range(w):
                                        cid = c0 + w0 + j
                                        t, k = sched["chunk_sched"][(h, g)][cid - run_chunk_start[ta, g]]
                                        gb = sched["bank_of_tile"][t]
                                        nc.tensor.matmul(
                                            bank_ap(t), P[:, j, :],
                                            msg[:, w0 + j, 0:H],
                                            start=False,
                                            stop=(cid == sched["bank_stop"][gb]))
                        for t in range(ta, tb):
                            nc.vector.tensor_scalar(
                                zh[:, t, 0:H], bank_ap(t),
                                dinv_sb[:, t:t + 1], 0.0, OP.mult, OP.max)

                if dbg:
                    nc.sync.dma_start(out=hdbg_p[L], in_=zh[:])
                if L < 2:
                    nc.sync.dma_start(
                        out=h_dram.rearrange("(t p) c -> p t c", p=128), in_=zh[:])
                    tc.strict_bb_all_engine_barrier()
                    nc.sync.dma_start_transpose(out=hT_sb[:], in_=h_dram[:])

            # ---- mean pooling ----
            with tc.tile_pool(name="psP", bufs=2, space="PSUM") as psP:
                pp = psP.tile([H, G], F32, tag="pp")
                for t in range(TPC):
                    nc.tensor.matmul(
                        pp[:], zh[:, t, 0:H], S_sb[:, t * G:(t + 1) * G],
                        start=(t == 0), stop=(t == TPC - 1))
                pool_sb = cpool.tile([H, G], F32, tag="pool_sb")
                nc.vector.tensor_copy(pool_sb[:], pp[:])
            nc.sync.dma_start(out=pool_loc[:], in_=pool_sb[:])
            if dbg:
                nc.sync.dma_start(out=pdbg_p[:], in_=pool_sb[:])
            tc.strict_bb_all_engine_barrier()
            nc.gpsimd.collective_compute(
                "AllReduce", OP.add, replica_groups=rg,
                ins=[pool_loc[:]], outs=[pool_full[:]])
            tc.strict_bb_all_engine_barrier()

            # ---- MLP head (replicated) ----
            with tc.tile_pool(name="psM", bufs=2, space="PSUM") as psM:
                pooled = cpool.tile([H, G], F32, tag="pooled")
                nc.sync.dma_start(pooled[:], pool_full[:])
                pooln = cpool.tile([H, G], F32, tag="pooln")
                nc.vector.tensor_tensor(pooln[:], pooled[:], cnti_sb[:], OP.mult)
                ps1 = psM.tile([H, G], F32, tag="ps1")
                nc.tensor.matmul(ps1[:], Wm1_sb[:], pooln[:], start=True, stop=True)
                z1 = cpool.tile([H, G], F32, tag="z1")
                nc.scalar.activation(z1[:], ps1[:],
                                     mybir.ActivationFunctionType.Relu,
                                     bias=bm1_sb[:, 0:1], scale=1.0)
                ps2 = psM.tile([OUT, G], F32, tag="ps2")
                nc.tensor.matmul(ps2[:], Wm2_sb[:], z1[:], start=True, stop=True)
                out_sb = cpool.tile([OUT, G], F32, tag="out_sb")
                nc.vector.tensor_scalar(out_sb[:], ps2[:], bm2_sb[:, 0:1], None, OP.add)
            nc.sync.dma_start(out=out_p[:], in_=out_sb[:])

    nc.compile()
    return nc


def run(inputs, cfg=REAL, via="hw", trace=False):
    """inputs: the reference.setup_inputs() dict (numpy). Returns [G, OUT] fp32."""
    sched, pci = prepare(inputs["x"], inputs["edge_index"], inputs["batch"], cfg)
    add_weight_inputs(pci, inputs["W1"], inputs["b1"], inputs["W2"], inputs["b2"],
                      inputs["W3"], inputs["b3"], inputs["Wm1"], inputs["bm1"],
                      inputs["Wm2"], inputs["bm2"])
    nc = build_program(cfg, sched)
    if via == "sim":
        from concourse.bass_interp import MultiCoreSim
        sim = MultiCoreSim(nc, num_cores=cfg.NCORES, require_finite=False,
                           require_nnan=False)
        for c, core in sim.cores.items():
            for k, v in pci[c].items():
                core.tensor(k)[:] = v
        sim.simulate(check_with_hw=False)
        out = np.array(sim.cores[0].tensor("out"))
        taps = None
        if getattr(cfg, "debug_taps", False):
            taps = {k: {c: np.array(sim.cores[c].tensor(k)) for c in sim.cores}
                    for k in ("zdbg", "hdbg", "pdbg")}
        return out.T.copy(), taps
    br = run_bass_kernel_spmd(nc, pci, list(range(cfg.NCORES)), trace=trace)
    out = br.results[0]["out"]
    return np.asarray(out, np.float32).T.copy(), br


def kernel(**inputs):
    inputs = {k: np.asarray(v) for k, v in inputs.items()}
    out, _ = run(inputs, REAL, via="hw")
    return out
